# revision 31
# baseline (speedup 1.0000x reference)
"""Trainium2 Bass kernel for nn_MAB_72911364817388 (dense transformer block).

Reference computation (per batch element b):
    q = Q @ Wq + bq ; k = K @ Wk + bk ; v = K @ Wv + bv        (1024x512 @ 512x512)
    scores = einsum("qhd,khd->hqk", qh, kh) / sqrt(512)
    scores = where(mask==0, -1e4, scores); attn = softmax(scores, axis=k)
    oh = qh + attn @ vh ; O = LN0(oh) ; O = O + relu(O @ Wo + bo) ; O = LN1(O)

Strategy: pure data-parallel over batch B=8 -> one batch element per core.

v3 design notes (vs the v2 baseline at ~163us):
- No mid-body tile_pool barriers: all pools are entered once at the top of
  the TileContext, PSUM is shared via two rotating tags ("sc" 2-bank x2,
  "u" 2-bank x2 = 8 banks), so phases and consecutive bodies overlap on
  the tile dependency graph alone.
- Explicit A/B double-buffering of all per-body SBUF state + a skewed
  software pipeline: body i emits DMA+projections+attention for i and
  interleaves the post stage (LN0/FC/LN1/store) of body i-1 into the PE
  stall gaps of attention (ACT exp is the attention pacer).
- fp8 (e4m3) DoubleRow matmuls for the k/v projections (K^T and Wk/Wv in
  fp8, 256-deep contraction pairs) and for attn@v (exp emits fp8 probs,
  vA in fp8, key-block pairs).  Scores / q-proj / FC stay bf16.
- The attention mask is folded into vA (masked v rows are zeroed by the
  mask multiply in the PSUM->SBUF copy; the softmax-denominator column
  holds the mask instead of ones), so exp needs no per-key-block bias and
  the -100-bias machinery disappears.
- ACT runs only exp (40x [128,1024]) + the LN rstd chain
  rstd = exp(-0.5*ln(var+eps)) -- ln and exp live in the same ACT table
  set, so there are zero per-body table swaps.  All PSUM->SBUF copies
  moved to Pool/DVE with the q/k biases folded in as tensor_scalar adds.
- Softmax denominators via DVE reciprocal_approx_fast (~5x faster than
  exact reciprocal), broadcast to 64 partitions with one [2,128]-const
  matmul per head pair.
- Constant inits (kT0 zero padding, ones, eps) hoisted out of the body.
"""

import numpy as np
import ml_dtypes

import concourse.bass as bass
import concourse.mybir as mybir
import concourse.tile as tile
from concourse import bacc, bass_utils

# Problem shapes (hardcoded per contract).
B = 8
NQ = NK = 1024
D = 512  # DQ = DK = DV
H = 8
HD = 64
P = 128
EPS = 1e-5
N_CORES = 8

DO = D // P   # 4  d-major partition groups
NO = NQ // P  # 8  key-major partition groups max
QC = NQ // 512  # 2 query free-dim chunks of 512

F32 = mybir.dt.float32
BF16 = mybir.dt.bfloat16
F8 = mybir.dt.float8e4
NPBF16 = ml_dtypes.bfloat16
NPF8 = ml_dtypes.float8_e4m3

AF = mybir.ActivationFunctionType
OP = mybir.AluOpType
DR = mybir.MatmulPerfMode.DoubleRow

# loop-unroll factor for the benchmark repeat loop
UNROLL = 8

def _find_ln_exp_set():
    try:
        from concourse.hw_specs import get_activation_tables
        tabs = get_activation_tables("Tonga4")
    except Exception:
        import json
        from pathlib import Path
        import concourse
        p = (Path(concourse.__file__).parent / "placeholder_act_info.json")
        tabs = {e["name"]: e for e in json.load(open(p))["act_func_sets"]}
    for i, name in enumerate(tabs):
        if name == "natural_log_exp_and_others":
            return i
    return 0


_ACT_SET_LN_EXP = _find_ln_exp_set()


def build_program(repeat: int = 1, apply_g0b0: bool = True,
                  apply_g1b1: bool = True, nkb: int = NO,
                  variant: str = "full"):
    nc = bacc.Bacc("TRN2", target_bir_lowering=False, debug=False,
                   num_devices=N_CORES)

    NKC = nkb * P
    nsm = 3 * DO + nkb + (2 * DO if apply_g0b0 else 0) \
        + (2 * DO if apply_g1b1 else 0)
    QTd = nc.dram_tensor("QT", [D, NQ], BF16, kind="ExternalInput").ap()
    KT8d = nc.dram_tensor("KT8", [D, NKC], F8, kind="ExternalInput").ap()
    WQOd = nc.dram_tensor("WQO", [2 * D, D], BF16,
                          kind="ExternalInput").ap()
    WKV8d = nc.dram_tensor("WKV8", [2 * D, D], F8,
                           kind="ExternalInput").ap()
    SMFd = nc.dram_tensor("SMF", [P, nsm], F32, kind="ExternalInput").ap()
    SMBd = nc.dram_tensor("SMB", [1, 2 * D], BF16,
                          kind="ExternalInput").ap()
    OTd = nc.dram_tensor("OT", [D, NQ], F32, kind="ExternalOutput").ap()

    import contextlib
    ctx = contextlib.ExitStack()
    with tile.TileContext(nc) as tc, ctx:
        sb = ctx.enter_context(tc.tile_pool(name="sb", bufs=1))
        ps = ctx.enter_context(tc.tile_pool(name="ps", bufs=1,
                                            space="PSUM"))

        # ---------- constants (one-time) ----------
        ones_bp = sb.tile([P, P], BF16)
        nc.vector.memset(ones_bp, 1.0)
        onesc = sb.tile([P, P], BF16)
        nc.vector.memset(onesc, 1.0 / HD)
        epsT = sb.tile([P, 1], F32)
        nc.vector.memset(epsT, EPS)

        # ---------- A/B per-body state ----------
        sets = []
        for par in range(2):
            S = {}
            S["WQO"] = sb.tile([P, 2 * DO, D], BF16, name=f"WQO{par}")
            S["WKV"] = sb.tile([P, 2 * DO, D], F8, name=f"WKV{par}")
            S["KT8"] = sb.tile([P, DO, NKC], F8, name=f"KT8{par}")
            S["QTb"] = sb.tile([P, DO, NQ], BF16, name=f"QTb{par}")
            S["SMF"] = sb.tile([P, nsm], F32, name=f"SMF{par}")
            S["SMB"] = sb.tile([1, 2 * D], BF16, name=f"SMB{par}")
            S["kT0"] = sb.tile([P, H, NKC], BF16, name=f"kT0{par}")
            S["qTb"] = sb.tile([P, DO, NQ], BF16, name=f"qTb{par}")
            S["vA"] = sb.tile([P, nkb, H * P], F8, name=f"vA{par}")
            S["ZT"] = sb.tile([P, DO, NQ], BF16, name=f"ZT{par}")
            S["XT"] = sb.tile([P, DO, NQ], BF16, name=f"XT{par}")
            S["Z2"] = sb.tile([P, DO, NQ], BF16, name=f"Z2{par}")
            sets.append(S)
        OTt = sb.tile([P, DO, NQ], F32)

        # one-time zero/init of state so the first (bogus) skewed post
        # stage computes on finite data, and kT0's inter-head zero rows
        # stay zero forever (bodies only write the head rows).
        for par in range(2):
            S = sets[par]
            for hh in range(2):
                nc.vector.memset(
                    S["kT0"][(1 - hh) * HD:(2 - hh) * HD, hh::2, :], 0.0)
        nc.vector.memset(OTt, 0.0)
        # pin the ACT table set that covers BOTH exp and ln so the
        # table-load pass never needs to thrash between per-func sets
        nc.scalar.add_instruction(mybir.InstLoadActFuncSet(
            name=nc.get_next_instruction_name(),
            act_func_set_id=_ACT_SET_LN_EXP, ins=[], outs=[]))

        def body(par, prev_par, first):
            _emit_body(nc, tc, sb, ps, sets[par],
                       None if first else sets[prev_par],
                       ones_bp, onesc, epsT, OTt,
                       QTd, KT8d, WQOd, WKV8d, SMFd, SMBd, OTd,
                       nkb, apply_g0b0, apply_g1b1, variant)

        def post_only(par):
            _emit_post(nc, sb, ps, sets[par], ones_bp, epsT, OTt,
                       OTd, apply_g0b0, apply_g1b1, variant,
                       parts=None)

        u = UNROLL
        if repeat <= u:
            for r in range(repeat):
                body(r % 2, (r - 1) % 2, r == 0)
            if variant not in ("dma", "attn"):
                post_only((repeat - 1) % 2)
        else:
            n_loop = (repeat // u - 1)
            # first group outside the loop (skew prologue)
            for r in range(u):
                body(r % 2, (r - 1) % 2, r == 0)
            if n_loop > 0:
                with tc.For_i(0, n_loop, 1,
                              hint_engines=(mybir.EngineType.PE,
                                            mybir.EngineType.Activation,
                                            mybir.EngineType.DVE,
                                            mybir.EngineType.SP,
                                            mybir.EngineType.Pool)):
                    for r in range(u):
                        body(r % 2, (r - 1) % 2, False)
            for r in range(repeat % u):
                body(r % 2, (r - 1) % 2, False)
            if variant not in ("dma", "attn"):
                post_only((repeat - 1) % 2)

    nc.compile()
    return nc


def _emit_post(nc, sb, ps, S, ones_bp, epsT, OTt, OTd,
               apply_g0b0, apply_g1b1, variant, parts):
    """LN0 -> FC(+relu+residual) -> LN1 -> store for a finished body.

    FC's matmuls run on raw ZT (rstd folded into the epilogue), so the
    PE work overlaps the LN0 variance/rstd chain; both qc's rstd come
    from one paired ln+exp on ACT.  When ``parts`` is a list, appends
    closures; when None, emits everything now."""
    do_dma = variant in ("full", "dma")

    ZT, XT, Z2 = S["ZT"], S["XT"], S["Z2"]
    SMF = S["SMF"]
    boT = SMF[:, 2 * DO:3 * DO]
    SMB = S["SMB"]
    wo1 = SMB[:, D:2 * D]
    WQO = S["WQO"]
    Wo_t = WQO[:, DO:2 * DO, :]
    fold_ln0 = not apply_g0b0
    nkb = S["_nkb"]
    off_g = 3 * DO + nkb
    if apply_g0b0:
        g0T = SMF[:, off_g:off_g + DO]
        b0T = SMF[:, off_g + DO:off_g + 2 * DO]
        off_g += 2 * DO
    else:
        g0T = b0T = None
    if apply_g1b1:
        g1T = SMF[:, off_g:off_g + DO]
        b1T = SMF[:, off_g + DO:off_g + 2 * DO]
    else:
        g1T = b1T = None

    st = {}

    def ln_stats(src_t, qc, label):
        """ones-matmul stats for column chunk qc; var lands in the
        shared varT[:, qc, :]."""
        s12 = ps.tile([P, 2, 512], F32, tag="u", bufs=2,
                      name=f"s12{label}{qc}")
        for ko in range(DO):
            nc.tensor.matmul(
                s12[:, 0, :], lhsT=ones_bp,
                rhs=src_t[:, ko, qc * 512:(qc + 1) * 512],
                start=(ko == 0), stop=(ko == DO - 1))
        for ko in range(DO):
            sq = sb.tile([P, 512], BF16, tag="sq", bufs=2,
                         name=f"sq{label}{qc}{ko}")
            nc.gpsimd.tensor_mul(
                out=sq, in0=src_t[:, ko, qc * 512:(qc + 1) * 512],
                in1=src_t[:, ko, qc * 512:(qc + 1) * 512])
            nc.tensor.matmul(s12[:, 1, :], lhsT=ones_bp, rhs=sq,
                             start=(ko == 0), stop=(ko == DO - 1))
        if label not in st:
            st[label] = {
                "varT": sb.tile([P, 2, 512], F32, tag="var", bufs=1,
                                name=f"varT{label}"),
                "rstdT": sb.tile([P, 2, 512], BF16, tag="rstd", bufs=1,
                                 name=f"rstdT{label}"),
                "muT": sb.tile([P, 2, 512], BF16, tag="mu", bufs=1,
                               name=f"muT{label}"),
                "negmu": [None, None],
            }
        d = st[label]
        mu_b = d["muT"][:, qc, :]
        nc.vector.tensor_scalar_mul(out=mu_b, in0=s12[:, 0, :],
                                    scalar1=1.0 / D)
        mu2 = sb.tile([P, 512], BF16, tag="mu2", bufs=2,
                      name=f"mu2{label}{qc}")
        nc.vector.tensor_mul(out=mu2, in0=mu_b, in1=mu_b)
        nc.vector.scalar_tensor_tensor(
            out=d["varT"][:, qc, :], in0=s12[:, 1, :], scalar=1.0 / D,
            in1=mu2, op0=OP.mult, op1=OP.subtract)
        if fold_ln0 and label == "a":
            ncr = sb.tile([1, 512], BF16, tag="negc", bufs=2,
                          name=f"negmu{qc}")
            nc.vector.tensor_scalar_mul(out=ncr, in0=s12[:1, 0, :],
                                        scalar1=-1.0 / D)
            d["negmu"][qc] = ncr

    def rstd_pair(label):
        d = st[label]
        lnv = sb.tile([P, 2, 512], F32, tag="lnv", bufs=1,
                      name=f"lnv{label}")
        nc.scalar.activation(lnv, d["varT"], AF.Ln, bias=epsT)
        nc.scalar.activation(d["rstdT"], lnv, AF.Exp, scale=-0.5)

    def g_mm(do):
        """FC matmuls on raw ZT (+ rank-1 -mu correction), no rstd dep."""
        fc = ps.tile([P, 2, 512], F32, tag="u", bufs=2, name=f"fc{do}")
        st["fc"][do] = fc
        for qc in range(QC):
            for ko in range(DO):
                nc.tensor.matmul(
                    fc[:, qc, :],
                    lhsT=Wo_t[:, ko, do * P:(do + 1) * P],
                    rhs=ZT[:, ko, qc * 512:(qc + 1) * 512],
                    start=(ko == 0),
                    stop=(ko == DO - 1 and not fold_ln0))
            if fold_ln0:
                nc.tensor.matmul(
                    fc[:, qc, :], lhsT=wo1[:, do * P:(do + 1) * P],
                    rhs=st["a"]["negmu"][qc], start=False, stop=True)

    def fc_tail(do):
        """fr = relu(G*rstd0 + bo); Z2 = XT + fr."""
        fc = st["fc"][do]
        rstdT = st["a"]["rstdT"]
        gm = sb.tile([P, 2, 512], BF16, tag="fr", bufs=2,
                     name=f"gm{do}")
        nc.vector.tensor_mul(out=gm, in0=fc, in1=rstdT)
        fr = sb.tile([P, 2, 512], BF16, tag="fr", bufs=2,
                     name=f"fr{do}")
        nc.vector.tensor_scalar(
            out=fr, in0=gm, scalar1=boT[:, do:do + 1],
            scalar2=0.0, op0=OP.add, op1=OP.max)
        nc.gpsimd.tensor_add(
            out=Z2[:, do, :], in0=fr.rearrange("p a b -> p (a b)"),
            in1=XT[:, do, :])

    def xt_do(do):
        rstdT = st["a"]["rstdT"]
        nc.gpsimd.tensor_mul(
            out=XT[:, do, :].rearrange("p (a b) -> p a b", b=512),
            in0=ZT[:, do, :].rearrange("p (a b) -> p a b", b=512),
            in1=rstdT)

    def ln0_general(qc):
        d = st["a"]
        mu_b = d["muT"][:, qc, :]
        rstd = d["rstdT"][:, qc, :]
        for ko in range(DO):
            ss = ZT[:, ko, qc * 512:(qc + 1) * 512]
            ds = XT[:, ko, qc * 512:(qc + 1) * 512]
            tm = sb.tile([P, 512], BF16, tag="sq", bufs=2,
                         name=f"tm0{qc}{ko}")
            nc.gpsimd.tensor_sub(out=tm, in0=ss, in1=mu_b)
            nc.vector.tensor_mul(out=ds, in0=tm, in1=rstd)
            nc.vector.tensor_scalar(
                out=ds, in0=ds, scalar1=g0T[:, ko:ko + 1],
                scalar2=b0T[:, ko:ko + 1], op0=OP.mult, op1=OP.add)

    def fc_general(do):
        fc = ps.tile([P, 2, 512], F32, tag="u", bufs=2, name=f"fc{do}")
        for qc in range(QC):
            for ko in range(DO):
                nc.tensor.matmul(
                    fc[:, qc, :],
                    lhsT=Wo_t[:, ko, do * P:(do + 1) * P],
                    rhs=XT[:, ko, qc * 512:(qc + 1) * 512],
                    start=(ko == 0), stop=(ko == DO - 1))
        fr = sb.tile([P, 2, 512], BF16, tag="fr", bufs=2,
                     name=f"fr{do}")
        nc.vector.tensor_scalar(
            out=fr, in0=fc, scalar1=boT[:, do:do + 1],
            scalar2=0.0, op0=OP.add, op1=OP.max)
        nc.gpsimd.tensor_add(
            out=Z2[:, do, :], in0=fr.rearrange("p a b -> p (a b)"),
            in1=XT[:, do, :])

    def ln1_norm(qc):
        d = st["b"]
        mu_b = d["muT"][:, qc, :]
        rstd = d["rstdT"][:, qc, :]
        cc = sb.tile([P, 512], BF16, tag="cc", bufs=2, name=f"cc{qc}")
        nc.vector.tensor_mul(out=cc, in0=mu_b, in1=rstd)
        for ko in range(DO):
            ss = Z2[:, ko, qc * 512:(qc + 1) * 512]
            ds = OTt[:, ko, qc * 512:(qc + 1) * 512]
            tm = sb.tile([P, 512], BF16, tag="sq", bufs=2,
                         name=f"tm1{qc}{ko}")
            nc.gpsimd.tensor_mul(out=tm, in0=ss, in1=rstd)
            if g1T is not None:
                nc.gpsimd.tensor_sub(out=ds, in0=tm, in1=cc)
                nc.vector.tensor_scalar(
                    out=ds, in0=ds, scalar1=g1T[:, ko:ko + 1],
                    scalar2=b1T[:, ko:ko + 1], op0=OP.mult, op1=OP.add)
            else:
                nc.gpsimd.tensor_sub(out=ds, in0=tm, in1=cc)
        if do_dma:
            od = OTd.rearrange("(o p) q -> p o q", p=P)
            nc.sync.dma_start(
                out=od[:, :, qc * 512:(qc + 1) * 512],
                in_=OTt[:, :, qc * 512:(qc + 1) * 512])

    st["fc"] = [None] * DO
    if fold_ln0:
        chunks = [
            lambda: (ln_stats(ZT, 0, "a"),),
            lambda: (ln_stats(ZT, 1, "a"), rstd_pair("a"),
                     g_mm(0), g_mm(1), g_mm(2), g_mm(3)),
            lambda: (xt_do(0), xt_do(1), xt_do(2), xt_do(3),
                     fc_tail(0), fc_tail(1), fc_tail(2), fc_tail(3),
                     ln_stats(Z2, 0, "b")),
            lambda: (ln_stats(Z2, 1, "b"), rstd_pair("b"),
                     ln1_norm(0), ln1_norm(1)),
        ]
    else:
        chunks = [
            lambda: (ln_stats(ZT, 0, "a"), ln_stats(ZT, 1, "a"),
                     rstd_pair("a"), ln0_general(0), ln0_general(1)),
            lambda: (fc_general(0), fc_general(1)),
            lambda: (fc_general(2), fc_general(3),
                     ln_stats(Z2, 0, "b")),
            lambda: (ln_stats(Z2, 1, "b"), rstd_pair("b"),
                     ln1_norm(0), ln1_norm(1)),
        ]
    if parts is None:
        for c in chunks:
            c()
    else:
        parts.extend(chunks)


def _emit_body(nc, tc, sb, ps, S, S_prev, ones_bp, onesc, epsT, OTt,
               QTd, KT8d, WQOd, WKV8d, SMFd, SMBd, OTd,
               nkb, apply_g0b0, apply_g1b1, variant):
    do_dma = variant in ("full", "dma", "attn")
    do_compute = variant in ("full", "compute", "attn")
    NKC = nkb * P
    SCALE = 1.0 / np.sqrt(np.float32(D))
    S["_nkb"] = nkb
    S["_OTd"] = OTd
    kchunks = []
    off = 0
    while off < NKC:
        w = min(512, NKC - off)
        kchunks.append((off, w))
        off += w

    # ---------- DMAs ----------
    if do_dma:
        nc.sync.dma_start(
            out=S["KT8"], in_=KT8d.rearrange("(o p) n -> p o n", p=P))
        nc.scalar.dma_start(
            out=S["QTb"], in_=QTd.rearrange("(o p) n -> p o n", p=P))
        nc.sync.dma_start(
            out=S["WKV"], in_=WKV8d.rearrange("(w p) n -> p w n", p=P))
        nc.sync.dma_start(
            out=S["WQO"], in_=WQOd.rearrange("(w p) n -> p w n", p=P))
        nc.sync.dma_start(out=S["SMF"], in_=SMFd)
        nc.sync.dma_start(out=S["SMB"], in_=SMBd)
    else:
        nc.vector.memset(S["KT8"], 0.01)
        nc.vector.memset(S["QTb"], 0.01)
        nc.vector.memset(S["WKV"], 0.01)
        nc.vector.memset(S["WQO"], 0.01)
        nc.vector.memset(S["SMF"], 0.01)
        nc.vector.memset(S["SMB"], 0.01)

    if not do_compute:
        od = OTd.rearrange("(o p) q -> p o q", p=P)
        for qc in range(QC):
            nc.sync.dma_start(
                out=od[:, :, qc * 512:(qc + 1) * 512],
                in_=OTt[:, :, qc * 512:(qc + 1) * 512])
        return

    SMF = S["SMF"]
    bqT = SMF[:, 0:DO]
    bkT = SMF[:, DO:2 * DO]
    mcol = SMF[:, 3 * DO:3 * DO + nkb]
    SMB = S["SMB"]
    bvR = SMB[:, 0:D]
    WQO, WKV, KT8, QTb = S["WQO"], S["WKV"], S["KT8"], S["QTb"]
    Wq_t = WQO[:, 0:DO, :]
    Wk8 = WKV[:, 0:DO, :]
    Wv8 = WKV[:, DO:2 * DO, :]
    kT0, qTb, vA = S["kT0"], S["qTb"], S["vA"]
    vAv = vA.rearrange("p k (h e) -> p k h e", e=P)
    ZT = S["ZT"]

    # post-stage chunks of the previous body, interleaved into attention
    parts = []
    if S_prev is not None and variant != "attn":
        _emit_post(nc, sb, ps, S_prev, ones_bp, epsT, OTt, OTd,
                   apply_g0b0, apply_g1b1, variant, parts=parts)
    while len(parts) < 4:
        parts.append(None)

    # ---------- projections (emitted per-hp inside the attention loop;
    # v right after hp0's scores so ACT starts exp'ing early) ----------
    def proj_do(do):
        pk = ps.tile([P, 2, 512], F32, tag="sc", bufs=2, name=f"pk{do}")
        pkf = pk.rearrange("p a b -> p (a b)")
        for g in range(2):
            for off, w in kchunks:
                nc.tensor.matmul(
                    pkf[:, off:off + w],
                    lhsT=WKV[:, 2 * g:2 * g + 2, do * P:(do + 1) * P],
                    rhs=KT8[:, 2 * g:2 * g + 2, off:off + w],
                    start=(g == 0), stop=(g == 1), perf_mode=DR)
        for hh in range(2):
            h = do * 2 + hh
            r0 = hh * HD
            nc.vector.tensor_scalar_add(
                out=kT0[r0:r0 + HD, h, :], in0=pkf[r0:r0 + HD, 0:NKC],
                scalar1=bkT[r0:r0 + HD, do:do + 1])
        pq = ps.tile([P, 2, 512], F32, tag="sc", bufs=2, name=f"pq{do}")
        for qc in range(QC):
            for ko in range(DO):
                nc.tensor.matmul(
                    pq[:, qc, :],
                    lhsT=Wq_t[:, ko, do * P:(do + 1) * P],
                    rhs=QTb[:, ko, qc * 512:(qc + 1) * 512],
                    start=(ko == 0), stop=(ko == DO - 1))
        nc.vector.tensor_scalar_add(
            out=qTb[:, do, :], in0=pq.rearrange("p a b -> p (a b)"),
            scalar1=bqT[:, do:do + 1])

    def vproj():
        for vp in range((nkb + 1) // 2):
            pv = ps.tile([P, 2, 512], F32, tag="sc", bufs=2,
                         name=f"pv{vp}")
            for j in range(2):
                no = 2 * vp + j
                if no >= nkb:
                    break
                for g in range(2):
                    nc.tensor.matmul(
                        pv[:, j, :],
                        lhsT=KT8[:, 2 * g:2 * g + 2, no * P:(no + 1) * P],
                        rhs=WKV[:, DO + 2 * g:DO + 2 * g + 2, :],
                        start=(g == 0), stop=False, perf_mode=DR)
                nc.tensor.matmul(pv[:, j, :], lhsT=ones_bp[:1, :],
                                 rhs=bvR, start=False, stop=True)
                pvv = pv[:, j, :].rearrange("p (h e) -> p h e", e=HD)
                nc.vector.tensor_scalar_mul(
                    out=vAv[:, no, 0::2, 0:HD], in0=pvv[:, 0::2, :],
                    scalar1=mcol[:, no:no + 1])
                nc.vector.tensor_scalar_mul(
                    out=vAv[:, no, 1::2, HD:P], in0=pvv[:, 1::2, :],
                    scalar1=mcol[:, no:no + 1])
                nc.gpsimd.tensor_copy(
                    out=vAv[:, no, 0::2, HD:P],
                    in_=mcol[:, no:no + 1].unsqueeze(2).broadcast_to(
                        (P, H // 2, HD)))
                nc.gpsimd.tensor_copy(
                    out=vAv[:, no, 1::2, 0:HD],
                    in_=mcol[:, no:no + 1].unsqueeze(2).broadcast_to(
                        (P, H // 2, HD)))

    # ---------- attention (head pairs hp), post chunks interleaved ----
    npair = nkb // 2
    tail = nkb % 2
    pts = {}
    Uss = {}

    def scores_chunk(hp, kbs):
        for kb in kbs:
            grp = kb // 2
            for hh in range(2):
                h = hp * 2 + hh
                sc = ps.tile([P, 2, 512], F32, tag="sc", bufs=2,
                             name=f"sc{hp}{kb}{hh}")
                for qc in range(QC):
                    nc.tensor.matmul(
                        sc[:, qc, :],
                        lhsT=kT0[:, h, kb * P:(kb + 1) * P],
                        rhs=qTb[:, hp, qc * 512:(qc + 1) * 512],
                        start=True, stop=True)
                if kb % 2 == 0:
                    pts[(hp, grp, hh)] = sb.tile(
                        [P, 2, 2, 512], F8, tag="pt", bufs=6,
                        name=f"pt{hp}{grp}{hh}")
                nc.scalar.activation(pts[(hp, grp, hh)][:, kb % 2, :, :],
                                     sc, AF.Exp, scale=SCALE)

    def emit_consumes(hp):
        Us = [ps.tile([P, 2 * 512], F32, tag="u", bufs=2,
                      name=f"U{hp}{hh}") for hh in range(2)]
        Uss[hp] = Us
        for g in range(npair):
            for hh in range(2):
                h = hp * 2 + hh
                for qc in range(QC):
                    nc.tensor.matmul(
                        Us[hh][:, qc * 512:(qc + 1) * 512],
                        lhsT=vA[:, 2 * g:2 * g + 2, h * P:(h + 1) * P],
                        rhs=pts[(hp, g, hh)][:, :, qc, :],
                        start=(g == 0),
                        stop=(g == npair - 1 and tail == 0),
                        perf_mode=DR)
        if tail:
            for hh in range(2):
                h = hp * 2 + hh
                for qc in range(QC):
                    nc.tensor.matmul(
                        Us[hh][:, qc * 512:(qc + 1) * 512],
                        lhsT=vA[:, nkb - 1, h * P:(h + 1) * P],
                        rhs=pts[(hp, npair, hh)][:, 0, qc, :],
                        start=(npair == 0), stop=True)

    def den_tail(hp):
        Us = Uss[hp]
        dencp = sb.tile([P, 2 * 512], BF16, tag="dencp", bufs=2,
                        name=f"dencp{hp}")
        nc.vector.tensor_copy(out=dencp[HD:P, :], in_=Us[0][HD:P, :])
        nc.vector.tensor_copy(out=dencp[0:HD, :], in_=Us[1][0:HD, :])
        B2 = ps.tile([P, 2, 512], F32, tag="sc", bufs=2, name=f"B2{hp}")
        for qc in range(QC):
            qs = slice(qc * 512, (qc + 1) * 512)
            nc.tensor.matmul(B2[0:HD, qc, :], lhsT=onesc[HD:P, 0:HD],
                             rhs=dencp[HD:P, qs], start=True, stop=True)
            nc.tensor.matmul(B2[HD:P, qc, :], lhsT=onesc[0:HD, HD:P],
                             rhs=dencp[0:HD, qs], start=True, stop=True)
        rec = sb.tile([P, 2 * 512], F32, tag="rec", bufs=2,
                      name=f"rec{hp}")
        nc.vector.reciprocal_approx_fast(
            out=rec, in_=B2.rearrange("p a b -> p (a b)"))
        nc.vector.tensor_mul(
            out=ZT[0:HD, hp, :], in0=Us[0][0:HD, :], in1=rec[0:HD, :])
        nc.vector.tensor_mul(
            out=ZT[HD:P, hp, :], in0=Us[1][HD:P, :], in1=rec[HD:P, :])
        nc.gpsimd.tensor_add(
            out=ZT[:, hp, :], in0=ZT[:, hp, :], in1=qTb[:, hp, :])

    proj_do(0)
    scores_chunk(0, list(range(nkb)))
    vproj()
    for hp in range(DO):
        emit_consumes(hp)
        nxt = hp + 1
        if nxt < DO:
            proj_do(nxt)
            scores_chunk(nxt, list(range(min(2, nkb))))
        den_tail(hp)
        if nxt < DO:
            scores_chunk(nxt, list(range(min(2, nkb), nkb)))
        if parts[hp] is not None:
            parts[hp]()
    if variant == "attn":
        # keep the attention live under DCE: dump ZT into OT's bytes
        odb = OTd.rearrange("(o p) q -> p o q", p=P).bitcast(BF16)
        nc.sync.dma_start(out=odb[:, :, 0:NQ], in_=ZT)


# ------------------------------------------------------------------
# host-side entry point
# ------------------------------------------------------------------
_CACHE = {}


def _get_program(repeat, apply_g0b0, apply_g1b1, nkb=NO):
    key = (repeat, apply_g0b0, apply_g1b1, nkb)
    if key not in _CACHE:
        _CACHE[key] = build_program(repeat, apply_g0b0, apply_g1b1, nkb)
    return _CACHE[key]


def compact_keys(K_b, mask_b, nkb):
    """Move unmasked keys to the front (order-preserving) and truncate to
    nkb*128 rows.  Softmax over keys is permutation-invariant and fully
    masked keys contribute exactly zero, so this is output-preserving as
    long as all unmasked keys survive the truncation."""
    nkc = nkb * P
    order = np.argsort(mask_b == 0, kind="stable")[:nkc]
    return (np.ascontiguousarray(K_b[order]),
            np.ascontiguousarray(mask_b[order]))


def pick_nkb(attention_mask):
    counts = (np.asarray(attention_mask) != 0).sum(axis=-1)
    return max(1, min(NO, int(-(-int(counts.max()) // P))))


def make_in_maps(inputs, nkb, apply_g0b0=None, apply_g1b1=None):
    if apply_g0b0 is None:
        apply_g0b0 = not (np.all(np.asarray(inputs["g0"]) == 1.0)
                          and np.all(np.asarray(inputs["b0"]) == 0.0))
    if apply_g1b1 is None:
        apply_g1b1 = not (np.all(np.asarray(inputs["g1"]) == 1.0)
                          and np.all(np.asarray(inputs["b1"]) == 0.0))

    def colT(name):
        return np.ascontiguousarray(
            np.asarray(inputs[name], np.float32).reshape(DO, P).T)

    wqo = np.concatenate([
        np.asarray(inputs["Wq"], NPBF16),
        np.asarray(inputs["Wo"], NPBF16)], axis=0)
    wkv8 = np.concatenate([
        np.asarray(inputs["Wk"], np.float32),
        np.asarray(inputs["Wv"], np.float32)], axis=0).astype(NPF8)
    smb = np.concatenate([
        np.asarray(inputs["bv"], NPBF16),
        np.asarray(np.asarray(inputs["Wo"], np.float32).sum(axis=0),
                   NPBF16)]).reshape(1, 2 * D)
    smf_cols = [colT("bq"), colT("bk"), colT("bo")]
    shared = {"WQO": wqo, "WKV8": wkv8, "SMB": smb}
    if apply_g0b0:
        g0b0 = [colT("g0"), colT("b0")]
    else:
        g0b0 = []
    if apply_g1b1:
        g1b1 = [colT("g1"), colT("b1")]
    else:
        g1b1 = []

    Q = np.asarray(inputs["Q"], np.float32)
    K = np.asarray(inputs["K"], np.float32)
    mask = np.asarray(inputs["attention_mask"], np.int32)
    in_maps = []
    for b in range(B):
        m = dict(shared)
        m["QT"] = np.ascontiguousarray(Q[b].T).astype(NPBF16)
        Kc, mc = compact_keys(K[b], mask[b], nkb)
        m["KT8"] = np.ascontiguousarray(Kc.T).astype(NPF8)
        mcolv = mc.astype(np.float32).reshape(nkb, P).T
        m["SMF"] = np.ascontiguousarray(np.concatenate(
            smf_cols + [mcolv] + g0b0 + g1b1, axis=1, dtype=np.float32))
        in_maps.append(m)
    return in_maps


def kernel(Q, K, attention_mask, Wq, bq, Wk, bk, Wv, bv, Wo, bo,
           g0, b0, g1, b1, _repeat=1):
    inputs = {
        "Q": Q, "K": K, "attention_mask": attention_mask,
        "Wq": Wq, "bq": bq, "Wk": Wk, "bk": bk, "Wv": Wv, "bv": bv,
        "Wo": Wo, "bo": bo, "g0": g0, "b0": b0, "g1": g1, "b1": b1,
    }
    apply_g0b0 = not (np.all(np.asarray(g0) == 1.0)
                      and np.all(np.asarray(b0) == 0.0))
    apply_g1b1 = not (np.all(np.asarray(g1) == 1.0)
                      and np.all(np.asarray(b1) == 0.0))
    nkb = pick_nkb(attention_mask)
    nc = _get_program(_repeat, apply_g0b0, apply_g1b1, nkb)
    in_maps = make_in_maps(inputs, nkb, apply_g0b0, apply_g1b1)

    res = bass_utils.run_bass_kernel_spmd(
        nc, in_maps, core_ids=list(range(N_CORES)), trace=False)

    out = np.empty((B, NQ, D), np.float32)
    for b in range(B):
        out[b] = res.results[b]["OT"].T
    return out


# revision 32
# speedup vs baseline: 1.0299x; 1.0299x over previous
"""Trainium2 Bass kernel for nn_MAB_72911364817388 (dense transformer block).

Reference computation (per batch element b):
    q = Q @ Wq + bq ; k = K @ Wk + bk ; v = K @ Wv + bv        (1024x512 @ 512x512)
    scores = einsum("qhd,khd->hqk", qh, kh) / sqrt(512)
    scores = where(mask==0, -1e4, scores); attn = softmax(scores, axis=k)
    oh = qh + attn @ vh ; O = LN0(oh) ; O = O + relu(O @ Wo + bo) ; O = LN1(O)

Strategy: pure data-parallel over batch B=8 -> one batch element per core.

v3 design notes (vs the v2 baseline at ~163us):
- No mid-body tile_pool barriers: all pools are entered once at the top of
  the TileContext, PSUM is shared via two rotating tags ("sc" 2-bank x2,
  "u" 2-bank x2 = 8 banks), so phases and consecutive bodies overlap on
  the tile dependency graph alone.
- Explicit A/B double-buffering of all per-body SBUF state + a skewed
  software pipeline: body i emits DMA+projections+attention for i and
  interleaves the post stage (LN0/FC/LN1/store) of body i-1 into the PE
  stall gaps of attention (ACT exp is the attention pacer).
- fp8 (e4m3) DoubleRow matmuls for the k/v projections (K^T and Wk/Wv in
  fp8, 256-deep contraction pairs) and for attn@v (exp emits fp8 probs,
  vA in fp8, key-block pairs).  Scores / q-proj / FC stay bf16.
- The attention mask is folded into vA (masked v rows are zeroed by the
  mask multiply in the PSUM->SBUF copy; the softmax-denominator column
  holds the mask instead of ones), so exp needs no per-key-block bias and
  the -100-bias machinery disappears.
- ACT runs only exp (40x [128,1024]) + the LN rstd chain
  rstd = exp(-0.5*ln(var+eps)) -- ln and exp live in the same ACT table
  set, so there are zero per-body table swaps.  All PSUM->SBUF copies
  moved to Pool/DVE with the q/k biases folded in as tensor_scalar adds.
- Softmax denominators via DVE reciprocal_approx_fast (~5x faster than
  exact reciprocal), broadcast to 64 partitions with one [2,128]-const
  matmul per head pair.
- Constant inits (kT0 zero padding, ones, eps) hoisted out of the body.
"""

import numpy as np
import ml_dtypes

import concourse.bass as bass
import concourse.mybir as mybir
import concourse.tile as tile
from concourse import bacc, bass_utils

# Problem shapes (hardcoded per contract).
B = 8
NQ = NK = 1024
D = 512  # DQ = DK = DV
H = 8
HD = 64
P = 128
EPS = 1e-5
N_CORES = 8

DO = D // P   # 4  d-major partition groups
NO = NQ // P  # 8  key-major partition groups max
QC = NQ // 512  # 2 query free-dim chunks of 512

F32 = mybir.dt.float32
BF16 = mybir.dt.bfloat16
F8 = mybir.dt.float8e4
NPBF16 = ml_dtypes.bfloat16
NPF8 = ml_dtypes.float8_e4m3

AF = mybir.ActivationFunctionType
OP = mybir.AluOpType
DR = mybir.MatmulPerfMode.DoubleRow

# loop-unroll factor for the benchmark repeat loop
UNROLL = 8

def _find_ln_exp_set():
    try:
        from concourse.hw_specs import get_activation_tables
        tabs = get_activation_tables("Tonga4")
    except Exception:
        import json
        from pathlib import Path
        import concourse
        p = (Path(concourse.__file__).parent / "placeholder_act_info.json")
        tabs = {e["name"]: e for e in json.load(open(p))["act_func_sets"]}
    for i, name in enumerate(tabs):
        if name == "natural_log_exp_and_others":
            return i
    return 0


_ACT_SET_LN_EXP = _find_ln_exp_set()


def build_program(repeat: int = 1, apply_g0b0: bool = True,
                  apply_g1b1: bool = True, nkb: int = NO,
                  variant: str = "full"):
    nc = bacc.Bacc("TRN2", target_bir_lowering=False, debug=False,
                   num_devices=N_CORES)

    NKC = nkb * P
    nsm = 3 * DO + nkb + (2 * DO if apply_g0b0 else 0) \
        + (2 * DO if apply_g1b1 else 0)
    QTd = nc.dram_tensor("QT", [D, NQ], BF16, kind="ExternalInput").ap()
    KT8d = nc.dram_tensor("KT8", [D, NKC], F8, kind="ExternalInput").ap()
    WQOd = nc.dram_tensor("WQO", [2 * D, D], BF16,
                          kind="ExternalInput").ap()
    WKV8d = nc.dram_tensor("WKV8", [2 * D, D], F8,
                           kind="ExternalInput").ap()
    SMFd = nc.dram_tensor("SMF", [P, nsm], F32, kind="ExternalInput").ap()
    SMBd = nc.dram_tensor("SMB", [1, 2 * D], BF16,
                          kind="ExternalInput").ap()
    OTd = nc.dram_tensor("OT", [D, NQ], F32, kind="ExternalOutput").ap()

    import contextlib
    ctx = contextlib.ExitStack()
    with tile.TileContext(nc) as tc, ctx:
        sb = ctx.enter_context(tc.tile_pool(name="sb", bufs=1))
        ps = ctx.enter_context(tc.tile_pool(name="ps", bufs=1,
                                            space="PSUM"))

        # ---------- constants (one-time) ----------
        ones_bp = sb.tile([P, P], BF16)
        nc.vector.memset(ones_bp, 1.0)
        onesc = sb.tile([P, P], BF16)
        nc.vector.memset(onesc, 1.0 / HD)
        epsT = sb.tile([P, 1], F32)
        nc.vector.memset(epsT, EPS)

        # ---------- A/B per-body state ----------
        sets = []
        for par in range(2):
            S = {}
            S["WQO"] = sb.tile([P, 2 * DO, D], BF16, name=f"WQO{par}")
            S["WKV"] = sb.tile([P, 2 * DO, D], F8, name=f"WKV{par}")
            S["KT8"] = sb.tile([P, DO, NKC], F8, name=f"KT8{par}")
            S["QTb"] = sb.tile([P, DO, NQ], BF16, name=f"QTb{par}")
            S["SMF"] = sb.tile([P, nsm], F32, name=f"SMF{par}")
            S["SMB"] = sb.tile([1, 2 * D], BF16, name=f"SMB{par}")
            S["kT0"] = sb.tile([P, H, NKC], BF16, name=f"kT0{par}")
            S["qTb"] = sb.tile([P, DO, NQ], BF16, name=f"qTb{par}")
            S["vA"] = sb.tile([P, nkb, H * P], F8, name=f"vA{par}")
            S["ZT"] = sb.tile([P, DO, NQ], BF16, name=f"ZT{par}")
            S["XT"] = sb.tile([P, DO, NQ], BF16, name=f"XT{par}")
            S["Z2"] = sb.tile([P, DO, NQ], BF16, name=f"Z2{par}")
            sets.append(S)
        OTt = sb.tile([P, DO, NQ], F32)

        # one-time zero/init of state so the first (bogus) skewed post
        # stage computes on finite data, and kT0's inter-head zero rows
        # stay zero forever (bodies only write the head rows).
        for par in range(2):
            S = sets[par]
            for hh in range(2):
                nc.vector.memset(
                    S["kT0"][(1 - hh) * HD:(2 - hh) * HD, hh::2, :], 0.0)
        nc.vector.memset(OTt, 0.0)
        # pin the ACT table set that covers BOTH exp and ln so the
        # table-load pass never needs to thrash between per-func sets
        nc.scalar.add_instruction(mybir.InstLoadActFuncSet(
            name=nc.get_next_instruction_name(),
            act_func_set_id=_ACT_SET_LN_EXP, ins=[], outs=[]))

        def body(par, prev_par, first):
            _emit_body(nc, tc, sb, ps, sets[par],
                       None if first else sets[prev_par],
                       ones_bp, onesc, epsT, OTt,
                       QTd, KT8d, WQOd, WKV8d, SMFd, SMBd, OTd,
                       nkb, apply_g0b0, apply_g1b1, variant)

        def post_only(par):
            _emit_post(nc, sb, ps, sets[par], ones_bp, epsT, OTt,
                       OTd, apply_g0b0, apply_g1b1, variant,
                       parts=None)

        u = UNROLL
        if repeat <= u:
            for r in range(repeat):
                body(r % 2, (r - 1) % 2, r == 0)
            if variant not in ("dma", "attn"):
                post_only((repeat - 1) % 2)
        else:
            n_loop = (repeat // u - 1)
            # first group outside the loop (skew prologue)
            for r in range(u):
                body(r % 2, (r - 1) % 2, r == 0)
            if n_loop > 0:
                with tc.For_i(0, n_loop, 1,
                              hint_engines=(mybir.EngineType.PE,
                                            mybir.EngineType.Activation,
                                            mybir.EngineType.DVE,
                                            mybir.EngineType.SP,
                                            mybir.EngineType.Pool)):
                    for r in range(u):
                        body(r % 2, (r - 1) % 2, False)
            for r in range(repeat % u):
                body(r % 2, (r - 1) % 2, False)
            if variant not in ("dma", "attn"):
                post_only((repeat - 1) % 2)

    nc.compile()
    return nc


def _emit_post(nc, sb, ps, S, ones_bp, epsT, OTt, OTd,
               apply_g0b0, apply_g1b1, variant, parts):
    """LN0 -> FC(+relu+residual) -> LN1 -> store for a finished body.

    FC's matmuls run on raw ZT (rstd folded into the epilogue), so the
    PE work overlaps the LN0 variance/rstd chain; both qc's rstd come
    from one paired ln+exp on ACT.  When ``parts`` is a list, appends
    closures; when None, emits everything now."""
    do_dma = variant in ("full", "dma")

    ZT, XT, Z2 = S["ZT"], S["XT"], S["Z2"]
    SMF = S["SMF"]
    boT = SMF[:, 2 * DO:3 * DO]
    SMB = S["SMB"]
    wo1 = SMB[:, D:2 * D]
    WQO = S["WQO"]
    Wo_t = WQO[:, DO:2 * DO, :]
    fold_ln0 = not apply_g0b0
    nkb = S["_nkb"]
    off_g = 3 * DO + nkb
    if apply_g0b0:
        g0T = SMF[:, off_g:off_g + DO]
        b0T = SMF[:, off_g + DO:off_g + 2 * DO]
        off_g += 2 * DO
    else:
        g0T = b0T = None
    if apply_g1b1:
        g1T = SMF[:, off_g:off_g + DO]
        b1T = SMF[:, off_g + DO:off_g + 2 * DO]
    else:
        g1T = b1T = None

    st = {}

    def ln_stats(src_t, qc, label):
        """ones-matmul stats for column chunk qc; var lands in the
        shared varT[:, qc, :]."""
        s12 = ps.tile([P, 2, 512], F32, tag="u", bufs=2,
                      name=f"s12{label}{qc}")
        for ko in range(DO):
            nc.tensor.matmul(
                s12[:, 0, :], lhsT=ones_bp,
                rhs=src_t[:, ko, qc * 512:(qc + 1) * 512],
                start=(ko == 0), stop=(ko == DO - 1))
        for ko in range(DO):
            sq = sb.tile([P, 512], BF16, tag="sq", bufs=2,
                         name=f"sq{label}{qc}{ko}")
            nc.gpsimd.tensor_mul(
                out=sq, in0=src_t[:, ko, qc * 512:(qc + 1) * 512],
                in1=src_t[:, ko, qc * 512:(qc + 1) * 512])
            nc.tensor.matmul(s12[:, 1, :], lhsT=ones_bp, rhs=sq,
                             start=(ko == 0), stop=(ko == DO - 1))
        if label not in st:
            st[label] = {
                "varT": sb.tile([P, 2, 512], F32, tag="var", bufs=1,
                                name=f"varT{label}"),
                "rstdT": sb.tile([P, 2, 512], BF16, tag="rstd", bufs=1,
                                 name=f"rstdT{label}"),
                "muT": sb.tile([P, 2, 512], BF16, tag="mu", bufs=1,
                               name=f"muT{label}"),
                "negmu": [None, None],
            }
        d = st[label]
        mu_b = d["muT"][:, qc, :]
        nc.vector.tensor_scalar_mul(out=mu_b, in0=s12[:, 0, :],
                                    scalar1=1.0 / D)
        mu2 = sb.tile([P, 512], BF16, tag="mu2", bufs=2,
                      name=f"mu2{label}{qc}")
        nc.vector.tensor_mul(out=mu2, in0=mu_b, in1=mu_b)
        nc.vector.scalar_tensor_tensor(
            out=d["varT"][:, qc, :], in0=s12[:, 1, :], scalar=1.0 / D,
            in1=mu2, op0=OP.mult, op1=OP.subtract)
        if fold_ln0 and label == "a":
            ncr = sb.tile([1, 512], BF16, tag="negc", bufs=2,
                          name=f"negmu{qc}")
            nc.vector.tensor_scalar_mul(out=ncr, in0=s12[:1, 0, :],
                                        scalar1=-1.0 / D)
            d["negmu"][qc] = ncr

    def rstd_pair(label):
        d = st[label]
        lnv = sb.tile([P, 2, 512], F32, tag="lnv", bufs=1,
                      name=f"lnv{label}")
        nc.scalar.activation(lnv, d["varT"], AF.Ln, bias=epsT)
        nc.scalar.activation(d["rstdT"], lnv, AF.Exp, scale=-0.5)

    def g_mm(do):
        """FC matmuls on raw ZT (+ rank-1 -mu correction), no rstd dep."""
        fc = ps.tile([P, 2, 512], F32, tag="u", bufs=2, name=f"fc{do}")
        st["fc"][do] = fc
        for qc in range(QC):
            for ko in range(DO):
                nc.tensor.matmul(
                    fc[:, qc, :],
                    lhsT=Wo_t[:, ko, do * P:(do + 1) * P],
                    rhs=ZT[:, ko, qc * 512:(qc + 1) * 512],
                    start=(ko == 0),
                    stop=(ko == DO - 1 and not fold_ln0))
            if fold_ln0:
                nc.tensor.matmul(
                    fc[:, qc, :], lhsT=wo1[:, do * P:(do + 1) * P],
                    rhs=st["a"]["negmu"][qc], start=False, stop=True)

    def fc_tail(do):
        """fr = relu(G*rstd0 + bo); Z2 = XT + fr."""
        fc = st["fc"][do]
        rstdT = st["a"]["rstdT"]
        gm = sb.tile([P, 2, 512], BF16, tag="fr", bufs=2,
                     name=f"gm{do}")
        nc.vector.tensor_mul(out=gm, in0=fc, in1=rstdT)
        fr = sb.tile([P, 2, 512], BF16, tag="fr", bufs=2,
                     name=f"fr{do}")
        nc.vector.tensor_scalar(
            out=fr, in0=gm, scalar1=boT[:, do:do + 1],
            scalar2=0.0, op0=OP.add, op1=OP.max)
        nc.gpsimd.tensor_add(
            out=Z2[:, do, :], in0=fr.rearrange("p a b -> p (a b)"),
            in1=XT[:, do, :])

    def xt_do(do):
        rstdT = st["a"]["rstdT"]
        nc.gpsimd.tensor_mul(
            out=XT[:, do, :].rearrange("p (a b) -> p a b", b=512),
            in0=ZT[:, do, :].rearrange("p (a b) -> p a b", b=512),
            in1=rstdT)

    def ln0_general(qc):
        d = st["a"]
        mu_b = d["muT"][:, qc, :]
        rstd = d["rstdT"][:, qc, :]
        for ko in range(DO):
            ss = ZT[:, ko, qc * 512:(qc + 1) * 512]
            ds = XT[:, ko, qc * 512:(qc + 1) * 512]
            tm = sb.tile([P, 512], BF16, tag="sq", bufs=2,
                         name=f"tm0{qc}{ko}")
            nc.gpsimd.tensor_sub(out=tm, in0=ss, in1=mu_b)
            nc.vector.tensor_mul(out=ds, in0=tm, in1=rstd)
            nc.vector.tensor_scalar(
                out=ds, in0=ds, scalar1=g0T[:, ko:ko + 1],
                scalar2=b0T[:, ko:ko + 1], op0=OP.mult, op1=OP.add)

    def fc_general(do):
        fc = ps.tile([P, 2, 512], F32, tag="u", bufs=2, name=f"fc{do}")
        for qc in range(QC):
            for ko in range(DO):
                nc.tensor.matmul(
                    fc[:, qc, :],
                    lhsT=Wo_t[:, ko, do * P:(do + 1) * P],
                    rhs=XT[:, ko, qc * 512:(qc + 1) * 512],
                    start=(ko == 0), stop=(ko == DO - 1))
        fr = sb.tile([P, 2, 512], BF16, tag="fr", bufs=2,
                     name=f"fr{do}")
        nc.vector.tensor_scalar(
            out=fr, in0=fc, scalar1=boT[:, do:do + 1],
            scalar2=0.0, op0=OP.add, op1=OP.max)
        nc.gpsimd.tensor_add(
            out=Z2[:, do, :], in0=fr.rearrange("p a b -> p (a b)"),
            in1=XT[:, do, :])

    def ln1_norm(qc):
        d = st["b"]
        mu_b = d["muT"][:, qc, :]
        rstd = d["rstdT"][:, qc, :]
        cc = sb.tile([P, 512], BF16, tag="cc", bufs=2, name=f"cc{qc}")
        nc.vector.tensor_mul(out=cc, in0=mu_b, in1=rstd)
        for ko in range(DO):
            ss = Z2[:, ko, qc * 512:(qc + 1) * 512]
            ds = OTt[:, ko, qc * 512:(qc + 1) * 512]
            tm = sb.tile([P, 512], BF16, tag="sq", bufs=2,
                         name=f"tm1{qc}{ko}")
            nc.gpsimd.tensor_mul(out=tm, in0=ss, in1=rstd)
            if g1T is not None:
                nc.gpsimd.tensor_sub(out=ds, in0=tm, in1=cc)
                nc.vector.tensor_scalar(
                    out=ds, in0=ds, scalar1=g1T[:, ko:ko + 1],
                    scalar2=b1T[:, ko:ko + 1], op0=OP.mult, op1=OP.add)
            else:
                nc.gpsimd.tensor_sub(out=ds, in0=tm, in1=cc)
        if do_dma:
            od = OTd.rearrange("(o p) q -> p o q", p=P)
            nc.sync.dma_start(
                out=od[:, :, qc * 512:(qc + 1) * 512],
                in_=OTt[:, :, qc * 512:(qc + 1) * 512])

    st["fc"] = [None] * DO
    if fold_ln0:
        chunks = [
            lambda: (ln_stats(ZT, 0, "a"),),
            lambda: (ln_stats(ZT, 1, "a"), rstd_pair("a"),
                     g_mm(0), g_mm(1), g_mm(2), g_mm(3)),
            lambda: (xt_do(0), xt_do(1), xt_do(2), xt_do(3),
                     fc_tail(0), fc_tail(1), fc_tail(2), fc_tail(3),
                     ln_stats(Z2, 0, "b")),
            lambda: (ln_stats(Z2, 1, "b"), rstd_pair("b"),
                     ln1_norm(0), ln1_norm(1)),
        ]
    else:
        chunks = [
            lambda: (ln_stats(ZT, 0, "a"), ln_stats(ZT, 1, "a"),
                     rstd_pair("a"), ln0_general(0), ln0_general(1)),
            lambda: (fc_general(0), fc_general(1)),
            lambda: (fc_general(2), fc_general(3),
                     ln_stats(Z2, 0, "b")),
            lambda: (ln_stats(Z2, 1, "b"), rstd_pair("b"),
                     ln1_norm(0), ln1_norm(1)),
        ]
    if parts is None:
        for c in chunks:
            c()
    else:
        parts.extend(chunks)


def _emit_body(nc, tc, sb, ps, S, S_prev, ones_bp, onesc, epsT, OTt,
               QTd, KT8d, WQOd, WKV8d, SMFd, SMBd, OTd,
               nkb, apply_g0b0, apply_g1b1, variant):
    do_dma = variant in ("full", "dma", "attn")
    do_compute = variant in ("full", "compute", "attn")
    NKC = nkb * P
    SCALE = 1.0 / np.sqrt(np.float32(D))
    S["_nkb"] = nkb
    S["_OTd"] = OTd
    kchunks = []
    off = 0
    while off < NKC:
        w = min(512, NKC - off)
        kchunks.append((off, w))
        off += w

    # ---------- DMAs ----------
    if do_dma:
        nc.scalar.dma_start(
            out=S["KT8"], in_=KT8d.rearrange("(o p) n -> p o n", p=P))
        nc.scalar.dma_start(
            out=S["QTb"], in_=QTd.rearrange("(o p) n -> p o n", p=P))
        nc.sync.dma_start(
            out=S["WKV"], in_=WKV8d.rearrange("(w p) n -> p w n", p=P))
        nc.sync.dma_start(
            out=S["WQO"], in_=WQOd.rearrange("(w p) n -> p w n", p=P))
        nc.sync.dma_start(out=S["SMF"], in_=SMFd)
        nc.sync.dma_start(out=S["SMB"], in_=SMBd)
    else:
        nc.vector.memset(S["KT8"], 0.01)
        nc.vector.memset(S["QTb"], 0.01)
        nc.vector.memset(S["WKV"], 0.01)
        nc.vector.memset(S["WQO"], 0.01)
        nc.vector.memset(S["SMF"], 0.01)
        nc.vector.memset(S["SMB"], 0.01)

    if not do_compute:
        od = OTd.rearrange("(o p) q -> p o q", p=P)
        for qc in range(QC):
            nc.sync.dma_start(
                out=od[:, :, qc * 512:(qc + 1) * 512],
                in_=OTt[:, :, qc * 512:(qc + 1) * 512])
        return

    SMF = S["SMF"]
    bqT = SMF[:, 0:DO]
    bkT = SMF[:, DO:2 * DO]
    mcol = SMF[:, 3 * DO:3 * DO + nkb]
    SMB = S["SMB"]
    bvR = SMB[:, 0:D]
    WQO, WKV, KT8, QTb = S["WQO"], S["WKV"], S["KT8"], S["QTb"]
    Wq_t = WQO[:, 0:DO, :]
    Wk8 = WKV[:, 0:DO, :]
    Wv8 = WKV[:, DO:2 * DO, :]
    kT0, qTb, vA = S["kT0"], S["qTb"], S["vA"]
    vAv = vA.rearrange("p k (h e) -> p k h e", e=P)
    ZT = S["ZT"]

    # post-stage chunks of the previous body, interleaved into attention
    parts = []
    if S_prev is not None and variant != "attn":
        _emit_post(nc, sb, ps, S_prev, ones_bp, epsT, OTt, OTd,
                   apply_g0b0, apply_g1b1, variant, parts=parts)
    while len(parts) < 4:
        parts.append(None)

    # ---------- projections (emitted per-hp inside the attention loop;
    # v right after hp0's scores so ACT starts exp'ing early) ----------
    def proj_do(do):
        pk = ps.tile([P, 2, 512], F32, tag="sc", bufs=2, name=f"pk{do}")
        pkf = pk.rearrange("p a b -> p (a b)")
        for g in range(2):
            for off, w in kchunks:
                nc.tensor.matmul(
                    pkf[:, off:off + w],
                    lhsT=WKV[:, 2 * g:2 * g + 2, do * P:(do + 1) * P],
                    rhs=KT8[:, 2 * g:2 * g + 2, off:off + w],
                    start=(g == 0), stop=(g == 1), perf_mode=DR)
        for hh in range(2):
            h = do * 2 + hh
            r0 = hh * HD
            nc.vector.tensor_scalar_add(
                out=kT0[r0:r0 + HD, h, :], in0=pkf[r0:r0 + HD, 0:NKC],
                scalar1=bkT[r0:r0 + HD, do:do + 1])
        pq = ps.tile([P, 2, 512], F32, tag="sc", bufs=2, name=f"pq{do}")
        for qc in range(QC):
            for ko in range(DO):
                nc.tensor.matmul(
                    pq[:, qc, :],
                    lhsT=Wq_t[:, ko, do * P:(do + 1) * P],
                    rhs=QTb[:, ko, qc * 512:(qc + 1) * 512],
                    start=(ko == 0), stop=(ko == DO - 1))
        nc.vector.tensor_scalar_add(
            out=qTb[:, do, :], in0=pq.rearrange("p a b -> p (a b)"),
            scalar1=bqT[:, do:do + 1])

    def vproj():
        for vp in range((nkb + 1) // 2):
            pv = ps.tile([P, 2, 512], F32, tag="sc", bufs=2,
                         name=f"pv{vp}")
            for j in range(2):
                no = 2 * vp + j
                if no >= nkb:
                    break
                for g in range(2):
                    nc.tensor.matmul(
                        pv[:, j, :],
                        lhsT=KT8[:, 2 * g:2 * g + 2, no * P:(no + 1) * P],
                        rhs=WKV[:, DO + 2 * g:DO + 2 * g + 2, :],
                        start=(g == 0), stop=False, perf_mode=DR)
                nc.tensor.matmul(pv[:, j, :], lhsT=ones_bp[:1, :],
                                 rhs=bvR, start=False, stop=True)
                pvv = pv[:, j, :].rearrange("p (h e) -> p h e", e=HD)
                nc.vector.tensor_scalar_mul(
                    out=vAv[:, no, 0::2, 0:HD], in0=pvv[:, 0::2, :],
                    scalar1=mcol[:, no:no + 1])
                nc.vector.tensor_scalar_mul(
                    out=vAv[:, no, 1::2, HD:P], in0=pvv[:, 1::2, :],
                    scalar1=mcol[:, no:no + 1])
                nc.gpsimd.tensor_copy(
                    out=vAv[:, no, 0::2, HD:P],
                    in_=mcol[:, no:no + 1].unsqueeze(2).broadcast_to(
                        (P, H // 2, HD)))
                nc.gpsimd.tensor_copy(
                    out=vAv[:, no, 1::2, 0:HD],
                    in_=mcol[:, no:no + 1].unsqueeze(2).broadcast_to(
                        (P, H // 2, HD)))

    # ---------- attention (head pairs hp), post chunks interleaved ----
    npair = nkb // 2
    tail = nkb % 2
    pts = {}
    Uss = {}

    def scores_chunk(hp, kbs):
        for kb in kbs:
            grp = kb // 2
            for hh in range(2):
                h = hp * 2 + hh
                sc = ps.tile([P, 2, 512], F32, tag="sc", bufs=2,
                             name=f"sc{hp}{kb}{hh}")
                for qc in range(QC):
                    nc.tensor.matmul(
                        sc[:, qc, :],
                        lhsT=kT0[:, h, kb * P:(kb + 1) * P],
                        rhs=qTb[:, hp, qc * 512:(qc + 1) * 512],
                        start=True, stop=True)
                if kb % 2 == 0:
                    pts[(hp, grp, hh)] = sb.tile(
                        [P, 2, 2, 512], F8, tag="pt", bufs=6,
                        name=f"pt{hp}{grp}{hh}")
                nc.scalar.activation(pts[(hp, grp, hh)][:, kb % 2, :, :],
                                     sc, AF.Exp, scale=SCALE)

    def emit_consumes(hp):
        Us = [ps.tile([P, 2 * 512], F32, tag="u", bufs=2,
                      name=f"U{hp}{hh}") for hh in range(2)]
        Uss[hp] = Us
        for g in range(npair):
            for hh in range(2):
                h = hp * 2 + hh
                for qc in range(QC):
                    nc.tensor.matmul(
                        Us[hh][:, qc * 512:(qc + 1) * 512],
                        lhsT=vA[:, 2 * g:2 * g + 2, h * P:(h + 1) * P],
                        rhs=pts[(hp, g, hh)][:, :, qc, :],
                        start=(g == 0),
                        stop=(g == npair - 1 and tail == 0),
                        perf_mode=DR)
        if tail:
            for hh in range(2):
                h = hp * 2 + hh
                for qc in range(QC):
                    nc.tensor.matmul(
                        Us[hh][:, qc * 512:(qc + 1) * 512],
                        lhsT=vA[:, nkb - 1, h * P:(h + 1) * P],
                        rhs=pts[(hp, npair, hh)][:, 0, qc, :],
                        start=(npair == 0), stop=True)

    def den_tail(hp):
        Us = Uss[hp]
        dencp = sb.tile([P, 2 * 512], BF16, tag="dencp", bufs=2,
                        name=f"dencp{hp}")
        nc.vector.tensor_copy(out=dencp[HD:P, :], in_=Us[0][HD:P, :])
        nc.vector.tensor_copy(out=dencp[0:HD, :], in_=Us[1][0:HD, :])
        B2 = ps.tile([P, 2, 512], F32, tag="sc", bufs=2, name=f"B2{hp}")
        for qc in range(QC):
            qs = slice(qc * 512, (qc + 1) * 512)
            nc.tensor.matmul(B2[0:HD, qc, :], lhsT=onesc[HD:P, 0:HD],
                             rhs=dencp[HD:P, qs], start=True, stop=True)
            nc.tensor.matmul(B2[HD:P, qc, :], lhsT=onesc[0:HD, HD:P],
                             rhs=dencp[0:HD, qs], start=True, stop=True)
        rec = sb.tile([P, 2 * 512], F32, tag="rec", bufs=2,
                      name=f"rec{hp}")
        nc.vector.reciprocal_approx_fast(
            out=rec, in_=B2.rearrange("p a b -> p (a b)"))
        nc.vector.tensor_mul(
            out=ZT[0:HD, hp, :], in0=Us[0][0:HD, :], in1=rec[0:HD, :])
        nc.vector.tensor_mul(
            out=ZT[HD:P, hp, :], in0=Us[1][HD:P, :], in1=rec[HD:P, :])
        nc.gpsimd.tensor_add(
            out=ZT[:, hp, :], in0=ZT[:, hp, :], in1=qTb[:, hp, :])

    proj_do(0)
    scores_chunk(0, list(range(nkb)))
    vproj()
    for hp in range(DO):
        emit_consumes(hp)
        nxt = hp + 1
        if nxt < DO:
            proj_do(nxt)
            scores_chunk(nxt, list(range(min(2, nkb))))
        den_tail(hp)
        if nxt < DO:
            scores_chunk(nxt, list(range(min(2, nkb), nkb)))
        if parts[hp] is not None:
            parts[hp]()
    if variant == "attn":
        # keep the attention live under DCE: dump ZT into OT's bytes
        odb = OTd.rearrange("(o p) q -> p o q", p=P).bitcast(BF16)
        nc.sync.dma_start(out=odb[:, :, 0:NQ], in_=ZT)


# ------------------------------------------------------------------
# host-side entry point
# ------------------------------------------------------------------
_CACHE = {}


def _get_program(repeat, apply_g0b0, apply_g1b1, nkb=NO):
    key = (repeat, apply_g0b0, apply_g1b1, nkb)
    if key not in _CACHE:
        _CACHE[key] = build_program(repeat, apply_g0b0, apply_g1b1, nkb)
    return _CACHE[key]


def compact_keys(K_b, mask_b, nkb):
    """Move unmasked keys to the front (order-preserving) and truncate to
    nkb*128 rows.  Softmax over keys is permutation-invariant and fully
    masked keys contribute exactly zero, so this is output-preserving as
    long as all unmasked keys survive the truncation."""
    nkc = nkb * P
    order = np.argsort(mask_b == 0, kind="stable")[:nkc]
    return (np.ascontiguousarray(K_b[order]),
            np.ascontiguousarray(mask_b[order]))


def pick_nkb(attention_mask):
    counts = (np.asarray(attention_mask) != 0).sum(axis=-1)
    return max(1, min(NO, int(-(-int(counts.max()) // P))))


def make_in_maps(inputs, nkb, apply_g0b0=None, apply_g1b1=None):
    if apply_g0b0 is None:
        apply_g0b0 = not (np.all(np.asarray(inputs["g0"]) == 1.0)
                          and np.all(np.asarray(inputs["b0"]) == 0.0))
    if apply_g1b1 is None:
        apply_g1b1 = not (np.all(np.asarray(inputs["g1"]) == 1.0)
                          and np.all(np.asarray(inputs["b1"]) == 0.0))

    def colT(name):
        return np.ascontiguousarray(
            np.asarray(inputs[name], np.float32).reshape(DO, P).T)

    wqo = np.concatenate([
        np.asarray(inputs["Wq"], NPBF16),
        np.asarray(inputs["Wo"], NPBF16)], axis=0)
    wkv8 = np.concatenate([
        np.asarray(inputs["Wk"], np.float32),
        np.asarray(inputs["Wv"], np.float32)], axis=0).astype(NPF8)
    smb = np.concatenate([
        np.asarray(inputs["bv"], NPBF16),
        np.asarray(np.asarray(inputs["Wo"], np.float32).sum(axis=0),
                   NPBF16)]).reshape(1, 2 * D)
    smf_cols = [colT("bq"), colT("bk"), colT("bo")]
    shared = {"WQO": wqo, "WKV8": wkv8, "SMB": smb}
    if apply_g0b0:
        g0b0 = [colT("g0"), colT("b0")]
    else:
        g0b0 = []
    if apply_g1b1:
        g1b1 = [colT("g1"), colT("b1")]
    else:
        g1b1 = []

    Q = np.asarray(inputs["Q"], np.float32)
    K = np.asarray(inputs["K"], np.float32)
    mask = np.asarray(inputs["attention_mask"], np.int32)
    in_maps = []
    for b in range(B):
        m = dict(shared)
        m["QT"] = np.ascontiguousarray(Q[b].T).astype(NPBF16)
        Kc, mc = compact_keys(K[b], mask[b], nkb)
        m["KT8"] = np.ascontiguousarray(Kc.T).astype(NPF8)
        mcolv = mc.astype(np.float32).reshape(nkb, P).T
        m["SMF"] = np.ascontiguousarray(np.concatenate(
            smf_cols + [mcolv] + g0b0 + g1b1, axis=1, dtype=np.float32))
        in_maps.append(m)
    return in_maps


def kernel(Q, K, attention_mask, Wq, bq, Wk, bk, Wv, bv, Wo, bo,
           g0, b0, g1, b1, _repeat=1):
    inputs = {
        "Q": Q, "K": K, "attention_mask": attention_mask,
        "Wq": Wq, "bq": bq, "Wk": Wk, "bk": bk, "Wv": Wv, "bv": bv,
        "Wo": Wo, "bo": bo, "g0": g0, "b0": b0, "g1": g1, "b1": b1,
    }
    apply_g0b0 = not (np.all(np.asarray(g0) == 1.0)
                      and np.all(np.asarray(b0) == 0.0))
    apply_g1b1 = not (np.all(np.asarray(g1) == 1.0)
                      and np.all(np.asarray(b1) == 0.0))
    nkb = pick_nkb(attention_mask)
    nc = _get_program(_repeat, apply_g0b0, apply_g1b1, nkb)
    in_maps = make_in_maps(inputs, nkb, apply_g0b0, apply_g1b1)

    res = bass_utils.run_bass_kernel_spmd(
        nc, in_maps, core_ids=list(range(N_CORES)), trace=False)

    out = np.empty((B, NQ, D), np.float32)
    for b in range(B):
        out[b] = res.results[b]["OT"].T
    return out


# revision 33
# speedup vs baseline: 1.0428x; 1.0125x over previous
"""Trainium2 Bass kernel for nn_MAB_72911364817388 (dense transformer block).

Reference computation (per batch element b):
    q = Q @ Wq + bq ; k = K @ Wk + bk ; v = K @ Wv + bv        (1024x512 @ 512x512)
    scores = einsum("qhd,khd->hqk", qh, kh) / sqrt(512)
    scores = where(mask==0, -1e4, scores); attn = softmax(scores, axis=k)
    oh = qh + attn @ vh ; O = LN0(oh) ; O = O + relu(O @ Wo + bo) ; O = LN1(O)

Strategy: pure data-parallel over batch B=8 -> one batch element per core.

v3 design notes (vs the v2 baseline at ~163us):
- No mid-body tile_pool barriers: all pools are entered once at the top of
  the TileContext, PSUM is shared via two rotating tags ("sc" 2-bank x2,
  "u" 2-bank x2 = 8 banks), so phases and consecutive bodies overlap on
  the tile dependency graph alone.
- Explicit A/B double-buffering of all per-body SBUF state + a skewed
  software pipeline: body i emits DMA+projections+attention for i and
  interleaves the post stage (LN0/FC/LN1/store) of body i-1 into the PE
  stall gaps of attention (ACT exp is the attention pacer).
- fp8 (e4m3) DoubleRow matmuls for the k/v projections (K^T and Wk/Wv in
  fp8, 256-deep contraction pairs) and for attn@v (exp emits fp8 probs,
  vA in fp8, key-block pairs).  Scores / q-proj / FC stay bf16.
- The attention mask is folded into vA (masked v rows are zeroed by the
  mask multiply in the PSUM->SBUF copy; the softmax-denominator column
  holds the mask instead of ones), so exp needs no per-key-block bias and
  the -100-bias machinery disappears.
- ACT runs only exp (40x [128,1024]) + the LN rstd chain
  rstd = exp(-0.5*ln(var+eps)) -- ln and exp live in the same ACT table
  set, so there are zero per-body table swaps.  All PSUM->SBUF copies
  moved to Pool/DVE with the q/k biases folded in as tensor_scalar adds.
- Softmax denominators via DVE reciprocal_approx_fast (~5x faster than
  exact reciprocal), broadcast to 64 partitions with one [2,128]-const
  matmul per head pair.
- Constant inits (kT0 zero padding, ones, eps) hoisted out of the body.
"""

import numpy as np
import ml_dtypes

import concourse.bass as bass
import concourse.mybir as mybir
import concourse.tile as tile
from concourse import bacc, bass_utils

# Problem shapes (hardcoded per contract).
B = 8
NQ = NK = 1024
D = 512  # DQ = DK = DV
H = 8
HD = 64
P = 128
EPS = 1e-5
N_CORES = 8

DO = D // P   # 4  d-major partition groups
NO = NQ // P  # 8  key-major partition groups max
QC = NQ // 512  # 2 query free-dim chunks of 512

F32 = mybir.dt.float32
BF16 = mybir.dt.bfloat16
F8 = mybir.dt.float8e4
NPBF16 = ml_dtypes.bfloat16
NPF8 = ml_dtypes.float8_e4m3

AF = mybir.ActivationFunctionType
OP = mybir.AluOpType
DR = mybir.MatmulPerfMode.DoubleRow

# loop-unroll factor for the benchmark repeat loop
UNROLL = 8

def _find_ln_exp_set():
    try:
        from concourse.hw_specs import get_activation_tables
        tabs = get_activation_tables("Tonga4")
    except Exception:
        import json
        from pathlib import Path
        import concourse
        p = (Path(concourse.__file__).parent / "placeholder_act_info.json")
        tabs = {e["name"]: e for e in json.load(open(p))["act_func_sets"]}
    for i, name in enumerate(tabs):
        if name == "natural_log_exp_and_others":
            return i
    return 0


_ACT_SET_LN_EXP = _find_ln_exp_set()


def build_program(repeat: int = 1, apply_g0b0: bool = True,
                  apply_g1b1: bool = True, nkb: int = NO,
                  variant: str = "full"):
    nc = bacc.Bacc("TRN2", target_bir_lowering=False, debug=False,
                   num_devices=N_CORES)

    NKC = nkb * P
    nsm = 3 * DO + nkb + (2 * DO if apply_g0b0 else 0) \
        + (2 * DO if apply_g1b1 else 0)
    QTd = nc.dram_tensor("QT", [D, NQ], BF16, kind="ExternalInput").ap()
    KT8d = nc.dram_tensor("KT8", [D, NKC], F8, kind="ExternalInput").ap()
    WQOd = nc.dram_tensor("WQO", [2 * D, D], BF16,
                          kind="ExternalInput").ap()
    WKV8d = nc.dram_tensor("WKV8", [2 * D, D], F8,
                           kind="ExternalInput").ap()
    SMFd = nc.dram_tensor("SMF", [P, nsm], F32, kind="ExternalInput").ap()
    SMBd = nc.dram_tensor("SMB", [1, 2 * D], BF16,
                          kind="ExternalInput").ap()
    OTd = nc.dram_tensor("OT", [D, NQ], F32, kind="ExternalOutput").ap()

    import contextlib
    ctx = contextlib.ExitStack()
    with tile.TileContext(nc) as tc, ctx:
        sb = ctx.enter_context(tc.tile_pool(name="sb", bufs=1))
        ps = ctx.enter_context(tc.tile_pool(name="ps", bufs=1,
                                            space="PSUM"))

        # ---------- constants (one-time) ----------
        ones_bp = sb.tile([P, P], BF16)
        nc.vector.memset(ones_bp, 1.0)
        onesc = sb.tile([P, P], BF16)
        nc.vector.memset(onesc, 1.0 / HD)
        epsT = sb.tile([P, 1], F32)
        nc.vector.memset(epsT, EPS)

        # ---------- A/B per-body state ----------
        sets = []
        for par in range(2):
            S = {}
            S["WQO"] = sb.tile([P, 2 * DO, D], BF16, name=f"WQO{par}")
            S["WKV"] = sb.tile([P, 2 * DO, D], F8, name=f"WKV{par}")
            S["KT8"] = sb.tile([P, DO, NKC], F8, name=f"KT8{par}")
            S["QTb"] = sb.tile([P, DO, NQ], BF16, name=f"QTb{par}")
            S["SMF"] = sb.tile([P, nsm], F32, name=f"SMF{par}")
            S["SMB"] = sb.tile([1, 2 * D], BF16, name=f"SMB{par}")
            S["kT0"] = sb.tile([P, H, NKC], BF16, name=f"kT0{par}")
            S["qTb"] = sb.tile([P, DO, NQ], BF16, name=f"qTb{par}")
            S["vA"] = sb.tile([P, nkb, H * P], F8, name=f"vA{par}")
            S["ZT"] = sb.tile([P, DO, NQ], BF16, name=f"ZT{par}")
            S["XT"] = sb.tile([P, DO, NQ], BF16, name=f"XT{par}")
            S["Z2"] = sb.tile([P, DO, NQ], BF16, name=f"Z2{par}")
            sets.append(S)
        OTt = sb.tile([P, DO, NQ], F32)

        # one-time zero/init of state so the first (bogus) skewed post
        # stage computes on finite data, and kT0's inter-head zero rows
        # stay zero forever (bodies only write the head rows).
        for par in range(2):
            S = sets[par]
            for hh in range(2):
                nc.vector.memset(
                    S["kT0"][(1 - hh) * HD:(2 - hh) * HD, hh::2, :], 0.0)
        nc.vector.memset(OTt, 0.0)
        # pin the ACT table set that covers BOTH exp and ln so the
        # table-load pass never needs to thrash between per-func sets
        nc.scalar.add_instruction(mybir.InstLoadActFuncSet(
            name=nc.get_next_instruction_name(),
            act_func_set_id=_ACT_SET_LN_EXP, ins=[], outs=[]))

        def body(par, prev_par, first):
            _emit_body(nc, tc, sb, ps, sets[par],
                       None if first else sets[prev_par],
                       ones_bp, onesc, epsT, OTt,
                       QTd, KT8d, WQOd, WKV8d, SMFd, SMBd, OTd,
                       nkb, apply_g0b0, apply_g1b1, variant)

        def post_only(par):
            _emit_post(nc, sb, ps, sets[par], ones_bp, epsT, OTt,
                       OTd, apply_g0b0, apply_g1b1, variant,
                       parts=None)

        u = UNROLL
        if repeat <= u:
            for r in range(repeat):
                body(r % 2, (r - 1) % 2, r == 0)
            if variant not in ("dma", "attn"):
                post_only((repeat - 1) % 2)
        else:
            n_loop = (repeat // u - 1)
            # first group outside the loop (skew prologue)
            for r in range(u):
                body(r % 2, (r - 1) % 2, r == 0)
            if n_loop > 0:
                with tc.For_i(0, n_loop, 1,
                              hint_engines=(mybir.EngineType.PE,
                                            mybir.EngineType.Activation,
                                            mybir.EngineType.DVE,
                                            mybir.EngineType.SP,
                                            mybir.EngineType.Pool)):
                    for r in range(u):
                        body(r % 2, (r - 1) % 2, False)
            for r in range(repeat % u):
                body(r % 2, (r - 1) % 2, False)
            if variant not in ("dma", "attn"):
                post_only((repeat - 1) % 2)

    nc.compile()
    return nc


def _emit_post(nc, sb, ps, S, ones_bp, epsT, OTt, OTd,
               apply_g0b0, apply_g1b1, variant, parts):
    """LN0 -> FC(+relu+residual) -> LN1 -> store for a finished body.

    FC's matmuls run on raw ZT (rstd folded into the epilogue), so the
    PE work overlaps the LN0 variance/rstd chain; both qc's rstd come
    from one paired ln+exp on ACT.  When ``parts`` is a list, appends
    closures; when None, emits everything now."""
    do_dma = variant in ("full", "dma")

    ZT, XT, Z2 = S["ZT"], S["XT"], S["Z2"]
    SMF = S["SMF"]
    boT = SMF[:, 2 * DO:3 * DO]
    SMB = S["SMB"]
    wo1 = SMB[:, D:2 * D]
    WQO = S["WQO"]
    Wo_t = WQO[:, DO:2 * DO, :]
    fold_ln0 = not apply_g0b0
    nkb = S["_nkb"]
    off_g = 3 * DO + nkb
    if apply_g0b0:
        g0T = SMF[:, off_g:off_g + DO]
        b0T = SMF[:, off_g + DO:off_g + 2 * DO]
        off_g += 2 * DO
    else:
        g0T = b0T = None
    if apply_g1b1:
        g1T = SMF[:, off_g:off_g + DO]
        b1T = SMF[:, off_g + DO:off_g + 2 * DO]
    else:
        g1T = b1T = None

    st = {}

    def ln_stats(src_t, qc, label):
        """ones-matmul stats for column chunk qc; var lands in the
        shared varT[:, qc, :]."""
        s12 = ps.tile([P, 2, 512], F32, tag="u", bufs=2,
                      name=f"s12{label}{qc}")
        for ko in range(DO):
            nc.tensor.matmul(
                s12[:, 0, :], lhsT=ones_bp,
                rhs=src_t[:, ko, qc * 512:(qc + 1) * 512],
                start=(ko == 0), stop=(ko == DO - 1))
        for ko in range(DO):
            sq = sb.tile([P, 512], BF16, tag="sq", bufs=2,
                         name=f"sq{label}{qc}{ko}")
            nc.gpsimd.tensor_mul(
                out=sq, in0=src_t[:, ko, qc * 512:(qc + 1) * 512],
                in1=src_t[:, ko, qc * 512:(qc + 1) * 512])
            nc.tensor.matmul(s12[:, 1, :], lhsT=ones_bp, rhs=sq,
                             start=(ko == 0), stop=(ko == DO - 1))
        if label not in st:
            st[label] = {
                "varT": sb.tile([P, 2, 512], F32, tag="var", bufs=1,
                                name=f"varT{label}"),
                "rstdT": sb.tile([P, 2, 512], BF16, tag="rstd", bufs=1,
                                 name=f"rstdT{label}"),
                "muT": sb.tile([P, 2, 512], BF16, tag="mu", bufs=1,
                               name=f"muT{label}"),
                "negmu": [None, None],
            }
        d = st[label]
        mu_b = d["muT"][:, qc, :]
        nc.vector.tensor_scalar_mul(out=mu_b, in0=s12[:, 0, :],
                                    scalar1=1.0 / D)
        mu2 = sb.tile([P, 512], BF16, tag="mu2", bufs=2,
                      name=f"mu2{label}{qc}")
        nc.vector.tensor_mul(out=mu2, in0=mu_b, in1=mu_b)
        nc.vector.scalar_tensor_tensor(
            out=d["varT"][:, qc, :], in0=s12[:, 1, :], scalar=1.0 / D,
            in1=mu2, op0=OP.mult, op1=OP.subtract)
        if fold_ln0 and label == "a":
            ncr = sb.tile([1, 512], BF16, tag="negc", bufs=2,
                          name=f"negmu{qc}")
            nc.vector.tensor_scalar_mul(out=ncr, in0=s12[:1, 0, :],
                                        scalar1=-1.0 / D)
            d["negmu"][qc] = ncr

    def rstd_pair(label):
        d = st[label]
        lnv = sb.tile([P, 2, 512], F32, tag="lnv", bufs=1,
                      name=f"lnv{label}")
        nc.scalar.activation(lnv, d["varT"], AF.Ln, bias=epsT)
        nc.scalar.activation(d["rstdT"], lnv, AF.Exp, scale=-0.5)

    def g_mm(do):
        """FC matmuls on raw ZT (+ rank-1 -mu correction), no rstd dep."""
        fc = ps.tile([P, 2, 512], F32, tag="u", bufs=2, name=f"fc{do}")
        st["fc"][do] = fc
        for qc in range(QC):
            for ko in range(DO):
                nc.tensor.matmul(
                    fc[:, qc, :],
                    lhsT=Wo_t[:, ko, do * P:(do + 1) * P],
                    rhs=ZT[:, ko, qc * 512:(qc + 1) * 512],
                    start=(ko == 0),
                    stop=(ko == DO - 1 and not fold_ln0))
            if fold_ln0:
                nc.tensor.matmul(
                    fc[:, qc, :], lhsT=wo1[:, do * P:(do + 1) * P],
                    rhs=st["a"]["negmu"][qc], start=False, stop=True)

    def fc_tail(do):
        """fr = relu(G*rstd0 + bo); Z2 = XT + fr."""
        fc = st["fc"][do]
        rstdT = st["a"]["rstdT"]
        gm = sb.tile([P, 2, 512], BF16, tag="fr", bufs=2,
                     name=f"gm{do}")
        nc.vector.tensor_mul(out=gm, in0=fc, in1=rstdT)
        fr = sb.tile([P, 2, 512], BF16, tag="fr", bufs=2,
                     name=f"fr{do}")
        nc.vector.tensor_scalar(
            out=fr, in0=gm, scalar1=boT[:, do:do + 1],
            scalar2=0.0, op0=OP.add, op1=OP.max)
        nc.gpsimd.tensor_add(
            out=Z2[:, do, :], in0=fr.rearrange("p a b -> p (a b)"),
            in1=XT[:, do, :])

    def xt_do(do):
        rstdT = st["a"]["rstdT"]
        nc.gpsimd.tensor_mul(
            out=XT[:, do, :].rearrange("p (a b) -> p a b", b=512),
            in0=ZT[:, do, :].rearrange("p (a b) -> p a b", b=512),
            in1=rstdT)

    def ln0_general(qc):
        d = st["a"]
        mu_b = d["muT"][:, qc, :]
        rstd = d["rstdT"][:, qc, :]
        for ko in range(DO):
            ss = ZT[:, ko, qc * 512:(qc + 1) * 512]
            ds = XT[:, ko, qc * 512:(qc + 1) * 512]
            tm = sb.tile([P, 512], BF16, tag="sq", bufs=2,
                         name=f"tm0{qc}{ko}")
            nc.gpsimd.tensor_sub(out=tm, in0=ss, in1=mu_b)
            nc.vector.tensor_mul(out=ds, in0=tm, in1=rstd)
            nc.vector.tensor_scalar(
                out=ds, in0=ds, scalar1=g0T[:, ko:ko + 1],
                scalar2=b0T[:, ko:ko + 1], op0=OP.mult, op1=OP.add)

    def fc_general(do):
        fc = ps.tile([P, 2, 512], F32, tag="u", bufs=2, name=f"fc{do}")
        for qc in range(QC):
            for ko in range(DO):
                nc.tensor.matmul(
                    fc[:, qc, :],
                    lhsT=Wo_t[:, ko, do * P:(do + 1) * P],
                    rhs=XT[:, ko, qc * 512:(qc + 1) * 512],
                    start=(ko == 0), stop=(ko == DO - 1))
        fr = sb.tile([P, 2, 512], BF16, tag="fr", bufs=2,
                     name=f"fr{do}")
        nc.vector.tensor_scalar(
            out=fr, in0=fc, scalar1=boT[:, do:do + 1],
            scalar2=0.0, op0=OP.add, op1=OP.max)
        nc.gpsimd.tensor_add(
            out=Z2[:, do, :], in0=fr.rearrange("p a b -> p (a b)"),
            in1=XT[:, do, :])

    def ln1_norm(qc):
        d = st["b"]
        mu_b = d["muT"][:, qc, :]
        rstd = d["rstdT"][:, qc, :]
        cc = sb.tile([P, 512], BF16, tag="cc", bufs=2, name=f"cc{qc}")
        nc.vector.tensor_mul(out=cc, in0=mu_b, in1=rstd)
        for ko in range(DO):
            ss = Z2[:, ko, qc * 512:(qc + 1) * 512]
            ds = OTt[:, ko, qc * 512:(qc + 1) * 512]
            tm = sb.tile([P, 512], BF16, tag="sq", bufs=2,
                         name=f"tm1{qc}{ko}")
            nc.gpsimd.tensor_mul(out=tm, in0=ss, in1=rstd)
            if g1T is not None:
                nc.gpsimd.tensor_sub(out=ds, in0=tm, in1=cc)
                nc.vector.tensor_scalar(
                    out=ds, in0=ds, scalar1=g1T[:, ko:ko + 1],
                    scalar2=b1T[:, ko:ko + 1], op0=OP.mult, op1=OP.add)
            else:
                nc.gpsimd.tensor_sub(out=ds, in0=tm, in1=cc)
        if do_dma:
            od = OTd.rearrange("(o p) q -> p o q", p=P)
            nc.sync.dma_start(
                out=od[:, :, qc * 512:(qc + 1) * 512],
                in_=OTt[:, :, qc * 512:(qc + 1) * 512])

    st["fc"] = [None] * DO
    if fold_ln0:
        chunks = [
            lambda: (ln_stats(ZT, 0, "a"),),
            lambda: (ln_stats(ZT, 1, "a"), rstd_pair("a"),
                     g_mm(0), g_mm(1), g_mm(2), g_mm(3)),
            lambda: (xt_do(0), xt_do(1), xt_do(2), xt_do(3),
                     fc_tail(0), fc_tail(1), fc_tail(2), fc_tail(3),
                     ln_stats(Z2, 0, "b")),
            lambda: (ln_stats(Z2, 1, "b"), rstd_pair("b"),
                     ln1_norm(0), ln1_norm(1)),
        ]
    else:
        chunks = [
            lambda: (ln_stats(ZT, 0, "a"), ln_stats(ZT, 1, "a"),
                     rstd_pair("a"), ln0_general(0), ln0_general(1)),
            lambda: (fc_general(0), fc_general(1)),
            lambda: (fc_general(2), fc_general(3),
                     ln_stats(Z2, 0, "b")),
            lambda: (ln_stats(Z2, 1, "b"), rstd_pair("b"),
                     ln1_norm(0), ln1_norm(1)),
        ]
    if parts is None:
        for c in chunks:
            c()
    else:
        parts.extend(chunks)


def _emit_body(nc, tc, sb, ps, S, S_prev, ones_bp, onesc, epsT, OTt,
               QTd, KT8d, WQOd, WKV8d, SMFd, SMBd, OTd,
               nkb, apply_g0b0, apply_g1b1, variant):
    do_dma = variant in ("full", "dma", "attn")
    do_compute = variant in ("full", "compute", "attn")
    NKC = nkb * P
    SCALE = 1.0 / np.sqrt(np.float32(D))
    S["_nkb"] = nkb
    S["_OTd"] = OTd
    kchunks = []
    off = 0
    while off < NKC:
        w = min(512, NKC - off)
        kchunks.append((off, w))
        off += w

    # ---------- DMAs ----------
    if do_dma:
        nc.scalar.dma_start(
            out=S["KT8"], in_=KT8d.rearrange("(o p) n -> p o n", p=P))
        nc.scalar.dma_start(
            out=S["QTb"], in_=QTd.rearrange("(o p) n -> p o n", p=P))
        nc.sync.dma_start(
            out=S["WKV"], in_=WKV8d.rearrange("(w p) n -> p w n", p=P))
        nc.sync.dma_start(
            out=S["WQO"], in_=WQOd.rearrange("(w p) n -> p w n", p=P))
        nc.sync.dma_start(out=S["SMF"], in_=SMFd)
        nc.sync.dma_start(out=S["SMB"], in_=SMBd)
    else:
        nc.vector.memset(S["KT8"], 0.01)
        nc.vector.memset(S["QTb"], 0.01)
        nc.vector.memset(S["WKV"], 0.01)
        nc.vector.memset(S["WQO"], 0.01)
        nc.vector.memset(S["SMF"], 0.01)
        nc.vector.memset(S["SMB"], 0.01)

    if not do_compute:
        od = OTd.rearrange("(o p) q -> p o q", p=P)
        for qc in range(QC):
            nc.sync.dma_start(
                out=od[:, :, qc * 512:(qc + 1) * 512],
                in_=OTt[:, :, qc * 512:(qc + 1) * 512])
        return

    SMF = S["SMF"]
    bqT = SMF[:, 0:DO]
    bkT = SMF[:, DO:2 * DO]
    mcol = SMF[:, 3 * DO:3 * DO + nkb]
    SMB = S["SMB"]
    bvR = SMB[:, 0:D]
    WQO, WKV, KT8, QTb = S["WQO"], S["WKV"], S["KT8"], S["QTb"]
    Wq_t = WQO[:, 0:DO, :]
    Wk8 = WKV[:, 0:DO, :]
    Wv8 = WKV[:, DO:2 * DO, :]
    kT0, qTb, vA = S["kT0"], S["qTb"], S["vA"]
    vAv = vA.rearrange("p k (h e) -> p k h e", e=P)
    ZT = S["ZT"]

    # post-stage chunks of the previous body, interleaved into attention
    parts = []
    if S_prev is not None and variant != "attn":
        _emit_post(nc, sb, ps, S_prev, ones_bp, epsT, OTt, OTd,
                   apply_g0b0, apply_g1b1, variant, parts=parts)
    while len(parts) < 4:
        parts.append(None)

    # ---------- projections (emitted per-hp inside the attention loop;
    # v right after hp0's scores so ACT starts exp'ing early) ----------
    def proj_do(do):
        pk = ps.tile([P, 2, 512], F32, tag="sc", bufs=2, name=f"pk{do}")
        pkf = pk.rearrange("p a b -> p (a b)")
        for g in range(2):
            for off, w in kchunks:
                nc.tensor.matmul(
                    pkf[:, off:off + w],
                    lhsT=WKV[:, 2 * g:2 * g + 2, do * P:(do + 1) * P],
                    rhs=KT8[:, 2 * g:2 * g + 2, off:off + w],
                    start=(g == 0), stop=(g == 1), perf_mode=DR)
        for hh in range(2):
            h = do * 2 + hh
            r0 = hh * HD
            nc.vector.tensor_scalar_add(
                out=kT0[r0:r0 + HD, h, :], in0=pkf[r0:r0 + HD, 0:NKC],
                scalar1=bkT[r0:r0 + HD, do:do + 1])
        pq = ps.tile([P, 2, 512], F32, tag="sc", bufs=2, name=f"pq{do}")
        for qc in range(QC):
            for ko in range(DO):
                nc.tensor.matmul(
                    pq[:, qc, :],
                    lhsT=Wq_t[:, ko, do * P:(do + 1) * P],
                    rhs=QTb[:, ko, qc * 512:(qc + 1) * 512],
                    start=(ko == 0), stop=(ko == DO - 1))
        nc.vector.tensor_scalar_add(
            out=qTb[:, do, :], in0=pq.rearrange("p a b -> p (a b)"),
            scalar1=bqT[:, do:do + 1])

    def vproj():
        for vp in range((nkb + 1) // 2):
            pv = ps.tile([P, 2, 512], F32, tag="sc", bufs=2,
                         name=f"pv{vp}")
            for j in range(2):
                no = 2 * vp + j
                if no >= nkb:
                    break
                for g in range(2):
                    nc.tensor.matmul(
                        pv[:, j, :],
                        lhsT=KT8[:, 2 * g:2 * g + 2, no * P:(no + 1) * P],
                        rhs=WKV[:, DO + 2 * g:DO + 2 * g + 2, :],
                        start=(g == 0), stop=False, perf_mode=DR)
                nc.tensor.matmul(pv[:, j, :], lhsT=ones_bp[:1, :],
                                 rhs=bvR, start=False, stop=True)
                pvv = pv[:, j, :].rearrange("p (h e) -> p h e", e=HD)
                nc.vector.tensor_scalar_mul(
                    out=vAv[:, no, 0::2, 0:HD], in0=pvv[:, 0::2, :],
                    scalar1=mcol[:, no:no + 1])
                nc.vector.tensor_scalar_mul(
                    out=vAv[:, no, 1::2, HD:P], in0=pvv[:, 1::2, :],
                    scalar1=mcol[:, no:no + 1])
                nc.gpsimd.tensor_copy(
                    out=vAv[:, no, 0::2, HD:P],
                    in_=mcol[:, no:no + 1].unsqueeze(2).broadcast_to(
                        (P, H // 2, HD)))
                nc.gpsimd.tensor_copy(
                    out=vAv[:, no, 1::2, 0:HD],
                    in_=mcol[:, no:no + 1].unsqueeze(2).broadcast_to(
                        (P, H // 2, HD)))

    # ---------- attention (head pairs hp), post chunks interleaved ----
    npair = nkb // 2
    tail = nkb % 2
    pts = {}
    Uss = {}

    def scores_chunk(hp, kbs):
        for kb in kbs:
            grp = kb // 2
            for hh in range(2):
                h = hp * 2 + hh
                sc = ps.tile([P, 2, 512], F32, tag="sc", bufs=2,
                             name=f"sc{hp}{kb}{hh}")
                for qc in range(QC):
                    nc.tensor.matmul(
                        sc[:, qc, :],
                        lhsT=kT0[:, h, kb * P:(kb + 1) * P],
                        rhs=qTb[:, hp, qc * 512:(qc + 1) * 512],
                        start=True, stop=True)
                if kb % 2 == 0:
                    pts[(hp, grp, hh)] = sb.tile(
                        [P, 2, 2, 512], F8, tag="pt", bufs=6,
                        name=f"pt{hp}{grp}{hh}")
                nc.scalar.activation(pts[(hp, grp, hh)][:, kb % 2, :, :],
                                     sc, AF.Exp, scale=SCALE)

    def emit_consumes(hp):
        Us = [ps.tile([P, 2 * 512], F32, tag="u", bufs=2,
                      name=f"U{hp}{hh}") for hh in range(2)]
        Uss[hp] = Us
        for g in range(npair):
            for hh in range(2):
                h = hp * 2 + hh
                for qc in range(QC):
                    nc.tensor.matmul(
                        Us[hh][:, qc * 512:(qc + 1) * 512],
                        lhsT=vA[:, 2 * g:2 * g + 2, h * P:(h + 1) * P],
                        rhs=pts[(hp, g, hh)][:, :, qc, :],
                        start=(g == 0),
                        stop=(g == npair - 1 and tail == 0),
                        perf_mode=DR)
        if tail:
            for hh in range(2):
                h = hp * 2 + hh
                for qc in range(QC):
                    nc.tensor.matmul(
                        Us[hh][:, qc * 512:(qc + 1) * 512],
                        lhsT=vA[:, nkb - 1, h * P:(h + 1) * P],
                        rhs=pts[(hp, npair, hh)][:, 0, qc, :],
                        start=(npair == 0), stop=True)

    def den_tail(hp):
        Us = Uss[hp]
        dencp = sb.tile([P, 2 * 512], BF16, tag="dencp", bufs=2,
                        name=f"dencp{hp}")
        nc.vector.tensor_copy(out=dencp[HD:P, :], in_=Us[0][HD:P, :])
        nc.vector.tensor_copy(out=dencp[0:HD, :], in_=Us[1][0:HD, :])
        B2 = ps.tile([P, 2, 512], F32, tag="sc", bufs=2, name=f"B2{hp}")
        for qc in range(QC):
            qs = slice(qc * 512, (qc + 1) * 512)
            nc.tensor.matmul(B2[0:HD, qc, :], lhsT=onesc[HD:P, 0:HD],
                             rhs=dencp[HD:P, qs], start=True, stop=True)
            nc.tensor.matmul(B2[HD:P, qc, :], lhsT=onesc[0:HD, HD:P],
                             rhs=dencp[0:HD, qs], start=True, stop=True)
        rec = sb.tile([P, 2 * 512], F32, tag="rec", bufs=2,
                      name=f"rec{hp}")
        nc.vector.reciprocal_approx_fast(
            out=rec, in_=B2.rearrange("p a b -> p (a b)"))
        nc.vector.tensor_mul(
            out=ZT[0:HD, hp, :], in0=Us[0][0:HD, :], in1=rec[0:HD, :])
        nc.vector.tensor_mul(
            out=ZT[HD:P, hp, :], in0=Us[1][HD:P, :], in1=rec[HD:P, :])
        nc.gpsimd.tensor_add(
            out=ZT[:, hp, :], in0=ZT[:, hp, :], in1=qTb[:, hp, :])

    proj_do(0)
    scores_chunk(0, list(range(min(2, nkb))))
    vproj()
    scores_chunk(0, list(range(min(2, nkb), nkb)))
    for hp in range(DO):
        emit_consumes(hp)
        nxt = hp + 1
        if nxt < DO:
            proj_do(nxt)
            scores_chunk(nxt, list(range(min(2, nkb))))
        den_tail(hp)
        if nxt < DO:
            scores_chunk(nxt, list(range(min(2, nkb), nkb)))
        if parts[hp] is not None:
            parts[hp]()
    if variant == "attn":
        # keep the attention live under DCE: dump ZT into OT's bytes
        odb = OTd.rearrange("(o p) q -> p o q", p=P).bitcast(BF16)
        nc.sync.dma_start(out=odb[:, :, 0:NQ], in_=ZT)


# ------------------------------------------------------------------
# host-side entry point
# ------------------------------------------------------------------
_CACHE = {}


def _get_program(repeat, apply_g0b0, apply_g1b1, nkb=NO):
    key = (repeat, apply_g0b0, apply_g1b1, nkb)
    if key not in _CACHE:
        _CACHE[key] = build_program(repeat, apply_g0b0, apply_g1b1, nkb)
    return _CACHE[key]


def compact_keys(K_b, mask_b, nkb):
    """Move unmasked keys to the front (order-preserving) and truncate to
    nkb*128 rows.  Softmax over keys is permutation-invariant and fully
    masked keys contribute exactly zero, so this is output-preserving as
    long as all unmasked keys survive the truncation."""
    nkc = nkb * P
    order = np.argsort(mask_b == 0, kind="stable")[:nkc]
    return (np.ascontiguousarray(K_b[order]),
            np.ascontiguousarray(mask_b[order]))


def pick_nkb(attention_mask):
    counts = (np.asarray(attention_mask) != 0).sum(axis=-1)
    return max(1, min(NO, int(-(-int(counts.max()) // P))))


def make_in_maps(inputs, nkb, apply_g0b0=None, apply_g1b1=None):
    if apply_g0b0 is None:
        apply_g0b0 = not (np.all(np.asarray(inputs["g0"]) == 1.0)
                          and np.all(np.asarray(inputs["b0"]) == 0.0))
    if apply_g1b1 is None:
        apply_g1b1 = not (np.all(np.asarray(inputs["g1"]) == 1.0)
                          and np.all(np.asarray(inputs["b1"]) == 0.0))

    def colT(name):
        return np.ascontiguousarray(
            np.asarray(inputs[name], np.float32).reshape(DO, P).T)

    wqo = np.concatenate([
        np.asarray(inputs["Wq"], NPBF16),
        np.asarray(inputs["Wo"], NPBF16)], axis=0)
    wkv8 = np.concatenate([
        np.asarray(inputs["Wk"], np.float32),
        np.asarray(inputs["Wv"], np.float32)], axis=0).astype(NPF8)
    smb = np.concatenate([
        np.asarray(inputs["bv"], NPBF16),
        np.asarray(np.asarray(inputs["Wo"], np.float32).sum(axis=0),
                   NPBF16)]).reshape(1, 2 * D)
    smf_cols = [colT("bq"), colT("bk"), colT("bo")]
    shared = {"WQO": wqo, "WKV8": wkv8, "SMB": smb}
    if apply_g0b0:
        g0b0 = [colT("g0"), colT("b0")]
    else:
        g0b0 = []
    if apply_g1b1:
        g1b1 = [colT("g1"), colT("b1")]
    else:
        g1b1 = []

    Q = np.asarray(inputs["Q"], np.float32)
    K = np.asarray(inputs["K"], np.float32)
    mask = np.asarray(inputs["attention_mask"], np.int32)
    in_maps = []
    for b in range(B):
        m = dict(shared)
        m["QT"] = np.ascontiguousarray(Q[b].T).astype(NPBF16)
        Kc, mc = compact_keys(K[b], mask[b], nkb)
        m["KT8"] = np.ascontiguousarray(Kc.T).astype(NPF8)
        mcolv = mc.astype(np.float32).reshape(nkb, P).T
        m["SMF"] = np.ascontiguousarray(np.concatenate(
            smf_cols + [mcolv] + g0b0 + g1b1, axis=1, dtype=np.float32))
        in_maps.append(m)
    return in_maps


def kernel(Q, K, attention_mask, Wq, bq, Wk, bk, Wv, bv, Wo, bo,
           g0, b0, g1, b1, _repeat=1):
    inputs = {
        "Q": Q, "K": K, "attention_mask": attention_mask,
        "Wq": Wq, "bq": bq, "Wk": Wk, "bk": bk, "Wv": Wv, "bv": bv,
        "Wo": Wo, "bo": bo, "g0": g0, "b0": b0, "g1": g1, "b1": b1,
    }
    apply_g0b0 = not (np.all(np.asarray(g0) == 1.0)
                      and np.all(np.asarray(b0) == 0.0))
    apply_g1b1 = not (np.all(np.asarray(g1) == 1.0)
                      and np.all(np.asarray(b1) == 0.0))
    nkb = pick_nkb(attention_mask)
    nc = _get_program(_repeat, apply_g0b0, apply_g1b1, nkb)
    in_maps = make_in_maps(inputs, nkb, apply_g0b0, apply_g1b1)

    res = bass_utils.run_bass_kernel_spmd(
        nc, in_maps, core_ids=list(range(N_CORES)), trace=False)

    out = np.empty((B, NQ, D), np.float32)
    for b in range(B):
        out[b] = res.results[b]["OT"].T
    return out
